# revision 23
# baseline (speedup 1.0000x reference)
"""Trainium2 Bass kernel for nn_BlockAttentionResidual (block attention + BitNet-style quantized MLP).

Sharding: sequence-block data parallelism, zero collectives. Block attention is
independent per 512-token block, so each of the 8 cores owns 1024 contiguous
tokens (2 blocks) of one batch element and runs the whole layer on them.
  core c -> batch c//4, tokens [(c%4)*1024, (c%4+1)*1024)

Weights are static parameters: ternarization (per-tensor mean|w| scale, exact
reference semantics) is host-side preprocessing; the ternary {-1,0,1} values are
exact in bf16.  Per-tensor dequant scalars fold into three constants shipped as
a tiny input tensor (exp-scale for attention, psum-eviction scales for o_proj /
ffn_down), so the device does no dequant bookkeeping: activations are
quantize-dequantized in one fused ACT+DVE pair per tile and all matmuls run on
bf16 operands with fp32 PSUM accumulation.

All DRAM layouts are pre-tiled host-side so every DMA line is >=1KB contiguous
per partition (the previous kernel was DMA-descriptor-bound: 600k descriptors
averaging 800B).
"""

import numpy as np
import ml_dtypes

import concourse.bass as bass
import concourse.mybir as mybir
import concourse.tile as tile
from concourse import bacc
from concourse.bass_utils import run_bass_kernel_spmd

F32 = mybir.dt.float32
BF16 = mybir.dt.bfloat16
AX = mybir.AxisListType
OP = mybir.AluOpType
ACTF = mybir.ActivationFunctionType

# model dims
H = 2048
NH = 16
HD = 128
NB = 8
INTER = 4096        # 2*H
EPS = 1e-5
THETA = 10000.0
B, S = 2, 4096
BT = 512            # tokens per attention block
NCORES = 8
R = 1024            # tokens per core
NT = R // 128       # 8 token tiles per core
NK = H // 128       # 16 k-tiles of the hidden dim
NKI = INTER // 128  # 32 k-tiles of the intermediate dim
MAGIC = np.float32(1.5 * 2 ** 23)   # fp32 round-to-nearest-even magic


def _quant_pair(nc, pool, src_ap, ncols, amax_ap, s_store, dq_store, out_bf,
                magic_ap, tag):
    """Quantize-dequantize src_ap [128, ncols] onto the int8 grid:
    out_bf = round(src*127/amax) * amax/127 in bf16.  amax_ap: [128,1] f32."""
    amc = pool.tile([128, 1], F32, tag=f"amc_{tag}")
    nc.vector.tensor_scalar_max(amc[:], amax_ap, 1e-5)
    rec = pool.tile([128, 1], F32, tag=f"rec_{tag}")
    nc.vector.reciprocal(rec[:], amc[:])
    nc.vector.tensor_scalar_mul(s_store, rec[:], 127.0)
    nc.vector.tensor_scalar_mul(dq_store, amc[:], 1.0 / 127.0)
    mg = pool.tile([128, ncols], F32, tag=f"mg_{tag}")
    nc.scalar.activation(mg[:], src_ap, ACTF.Identity, bias=magic_ap, scale=s_store)
    nc.vector.tensor_scalar(out_bf, mg[:], float(MAGIC), dq_store, OP.subtract,
                            OP.mult)


def build_program():
    nc = bacc.Bacc(None, target_bir_lowering=False)

    # ---- I/O (all layouts pre-tiled on host) ----
    x_in = nc.declare_dram_parameter("x_sh", [R, H], F32, isOutput=False)
    cs_in = nc.declare_dram_parameter("cs_sh", [128, 2, NT, 256], BF16, isOutput=False)
    anw_in = nc.declare_dram_parameter("attn_norm_w", [H], F32, isOutput=False)
    fnw_in = nc.declare_dram_parameter("ffn_norm_w", [H], F32, isOutput=False)
    consts_in = nc.declare_dram_parameter("consts", [8], F32, isOutput=False)
    # weight strips: [strip, 128 kpart, n_ktiles, 512 outcols] bf16
    wqkv_in = nc.declare_dram_parameter("wqkv_sh", [12, 128, NK, 512], BF16, isOutput=False)
    wo_in = nc.declare_dram_parameter("wo_sh", [4, 128, NK, 512], BF16, isOutput=False)
    wup_in = nc.declare_dram_parameter("wup_sh", [16, 128, NK, 512], BF16, isOutput=False)
    # ffn_down strips split in two k-halves: [strip, half, 128, 16, 512]
    wdn_in = nc.declare_dram_parameter("wdn_sh", [4, 2, 128, 16, 512], BF16, isOutput=False)
    out_d = nc.declare_dram_parameter("out_sh", [R, H], F32, isOutput=True)

    # ---- internal DRAM scratch ----
    x1_d = nc.dram_tensor("x1_d", [NT, 128, H], F32)
    act_d = nc.dram_tensor("act_d", [NT, 128, INTER], BF16)      # silu(g)*v rows
    act2_d = nc.dram_tensor("act2_d", [NT, 128, NKI, 128], BF16)  # quantized, transposed

    with tile.TileContext(nc) as tc:
        perm = tc.alloc_tile_pool(name="perm", bufs=1)
        magic_t = perm.tile([128, 1], F32)
        nc.vector.memset(magic_t[:], float(MAGIC))
        magic_ap = magic_t[:]
        consts_b = perm.tile([128, 8], F32)
        ap0 = consts_in[:]
        nc.gpsimd.dma_start(out=consts_b[:], in_=bass.AP(
            tensor=ap0.tensor, offset=ap0.offset, ap=[[0, 128]] + list(ap0.ap)))
        c_att = consts_b[:, 0:1]   # dqw_qkv^2 * HD^-0.5
        c_o = consts_b[:, 1:2]     # dqw_qkv * dqw_o
        c_dn = consts_b[:, 2:3]    # dqw_dn
        # per-token quant scales (s = 127/amax, dq = amax/127)
        s1 = perm.tile([128, NT], F32)
        dq1 = perm.tile([128, NT], F32)
        sc = perm.tile([128, NT], F32)
        dqc = perm.tile([128, NT], F32)
        sa = perm.tile([128, NT], F32)
        dqa = perm.tile([128, NT], F32)
        amax_av = perm.tile([128, NT], F32)
        nc.vector.memset(amax_av[:], 0.0)

        ssq_parts = perm.tile([128, NT, 4], F32)

        cs_pool = tc.alloc_tile_pool(name="cs_pool", bufs=1)
        cs_b = cs_pool.tile([128, 2, NT, 256], BF16)
        nc.gpsimd.dma_start(cs_b[:], cs_in[:])

        ctx_pool = tc.alloc_tile_pool(name="ctx_pool", bufs=1)
        ctx_sb = ctx_pool.tile([128, NT, NH, 128], BF16)
        slot1 = tc.alloc_tile_pool(name="slot1", bufs=1)
        xqT = slot1.tile([128, NK, NT, 128], BF16)

        # ---------- P1: attn rmsnorm + act-qdq + transpose ----------
        with tc.tile_pool(name="npool", bufs=2) as npool, \
             tc.tile_pool(name="nwpool", bufs=1) as nwpool:
            anw_b = nwpool.tile([128, H], F32, tag="normw")
            ap1 = anw_in[:]
            nc.gpsimd.dma_start(out=anw_b[:], in_=bass.AP(
                tensor=ap1.tensor, offset=ap1.offset, ap=[[0, 128]] + list(ap1.ap)))
            for t in range(NT):
                xt = npool.tile([128, H], F32, tag="xt")
                nc.gpsimd.dma_start(xt[:], x_in[t * 128:(t + 1) * 128, :])
                ssq = npool.tile([128, 1], F32, tag="ssq")
                junk = npool.tile([128, H], BF16, tag="njunk")
                nc.scalar.activation(junk[:], xt[:], ACTF.Square, accum_out=ssq[:])
                msq = npool.tile([128, 1], F32, tag="msq")
                nc.vector.tensor_scalar(msq[:], ssq[:], 1.0 / H, EPS, OP.mult, OP.add)
                sd = npool.tile([128, 1], F32, tag="sd")
                nc.scalar.activation(sd[:], msq[:], ACTF.Sqrt)
                rstd = npool.tile([128, 1], F32, tag="rstd")
                nc.vector.reciprocal(rstd[:], sd[:])
                h_t = npool.tile([128, H], F32, tag="h_t")
                nc.vector.tensor_scalar_mul(h_t[:], xt[:], rstd[:])
                nc.vector.tensor_tensor(h_t[:], h_t[:], anw_b[:], OP.mult)
                amax = npool.tile([128, 1], F32, tag="amax_n1")
                nc.vector.tensor_reduce(amax[:], h_t[:], AX.X, OP.max,
                                        apply_absolute_value=True)
                xq = npool.tile([128, H], BF16, tag="xq")
                _quant_pair(nc, npool, h_t[:], H, amax[:], s1[:, t:t + 1],
                            dq1[:, t:t + 1], xq[:], magic_ap, "n1")
                nc.sync.dma_start_transpose(xqT[:, :, t, :], xq[:])

        # ---------- P2+P3: qkv matmul + rope interleaved with block attention ----------
        # strips ordered q_g,k_g,v_g per 4-head group g; attention for group g
        # runs right after its three strips, overlapping later groups' matmuls.
        with tc.tile_pool(name="qkwpool", bufs=2) as qkwpool, \
             tc.tile_pool(name="ropool", bufs=3) as ropool, \
             tc.tile_pool(name="qkpool", bufs=2) as qkpool, \
             tc.tile_pool(name="apool", bufs=2) as apool, \
             tc.tile_pool(name="qkv_psum", bufs=3, space="PSUM") as psum_mm, \
             tc.tile_pool(name="at_psum", bufs=3, space="PSUM", side="right") as psum_at, \
             tc.tile_pool(name="ctx_psum", bufs=2, space="PSUM", side="right") as psum_cx:
            for g in range(4):
                qTg = qkpool.tile([128, 4, NT, 128], BF16, tag="qTg")
                kTg = qkpool.tile([128, 4, NT, 128], BF16, tag="kTg")
                v_g = qkpool.tile([128, NT, 4, 132], BF16, tag="vg")
                nc.vector.memset(v_g[:, :, :, 128:129], 1.0)
                for kind in range(3):
                    src = g + 4 * kind
                    wst = qkwpool.tile([128, NK, 512], BF16, tag="w_qkv")
                    nc.gpsimd.dma_start(wst[:], wqkv_in[src, :, :, :])
                    for t in range(NT):
                        ps = psum_mm.tile([128, 512], F32, tag="ps_qkv")
                        for kk in range(NK):
                            nc.tensor.matmul(ps[:], xqT[:, kk, t, :], wst[:, kk, :],
                                             start=(kk == 0), stop=(kk == NK - 1))
                        if kind == 2:
                            nc.scalar.copy(v_g[:, t, :, 0:128],
                                           ps[:].rearrange("p (c f) -> p c f", c=4))
                        else:
                            qsc = ropool.tile([128, 4, 128], BF16, tag="qsc")
                            nc.scalar.copy(qsc[:], ps[:].rearrange("p (c f) -> p c f", c=4))
                            p1, p2 = qsc[:, :, 0:64], qsc[:, :, 64:128]
                            cosd = cs_b[:, 0, t, :].rearrange("p (c f) -> p c f", c=4)
                            sind = cs_b[:, 1, t, :].rearrange("p (c f) -> p c f", c=4)
                            t1 = ropool.tile([128, 4, 64], F32, tag="rt1")
                            t2 = ropool.tile([128, 4, 64], F32, tag="rt2")
                            rot = ropool.tile([128, 4, 128], BF16, tag="rot")
                            nc.vector.tensor_tensor(t1[:], p1, cosd, OP.mult)
                            nc.gpsimd.tensor_tensor(t2[:], p2, sind, OP.mult)
                            nc.vector.tensor_tensor(rot[:, :, 0:64], t1[:], t2[:], OP.subtract)
                            nc.vector.tensor_tensor(t1[:], p2, cosd, OP.mult)
                            nc.gpsimd.tensor_tensor(t2[:], p1, sind, OP.mult)
                            nc.vector.tensor_tensor(rot[:, :, 64:128], t1[:], t2[:], OP.add)
                            dst = qTg if kind == 0 else kTg
                            nc.sync.dma_start_transpose(
                                dst[:, :, t, :], rot[:].rearrange("p c f -> p (c f)"))
                for blk in range(2):
                    for hl in range(4):
                        expT = [None] * 4
                        for kt in range(4):
                            qn = 512 - kt * 128
                            pss = psum_at.tile([128, 512], F32, tag="ps_sc")
                            nc.tensor.matmul(
                                pss[:, 0:qn],
                                kTg[:, hl, blk * 4 + kt, :],
                                qTg[:, hl, blk * 4:(blk + 1) * 4, :]
                                .rearrange("p c f -> p (c f)")[:, kt * 128:512],
                                start=True, stop=True)
                            ex = apool.tile([128, 512], BF16, tag=f"expT{kt}")
                            nc.scalar.activation(ex[:, 0:qn], pss[:, 0:qn], ACTF.Exp,
                                                 scale=c_att)
                            nc.gpsimd.affine_select(
                                out=ex[:, 0:128], in_=ex[:, 0:128],
                                compare_op=OP.is_ge, fill=0.0,
                                base=0, pattern=[[1, 128]], channel_multiplier=-1)
                            expT[kt] = ex
                        for qt in range(4):
                            psc = psum_cx.tile([128, 132], F32, tag="ps_ctx")
                            for kt in range(qt + 1):
                                nc.tensor.matmul(psc[:, 0:129],
                                                 expT[kt][:, (qt - kt) * 128:(qt - kt) * 128 + 128],
                                                 v_g[:, blk * 4 + kt, hl, 0:129],
                                                 start=(kt == 0), stop=(kt == qt))
                            rl = apool.tile([128, 1], F32, tag="rl")
                            nc.vector.reciprocal(rl[:], psc[:, 128:129])
                            nc.vector.tensor_scalar_mul(
                                ctx_sb[:, blk * 4 + qt, 4 * g + hl, :],
                                psc[:, 0:128], rl[:])
        slot1.release()

        # ---------- P4: ctx act-qdq + transpose ----------
        ctxq_pool = tc.alloc_tile_pool(name="ctxq_pool", bufs=1, side="right")
        ctxqT = ctxq_pool.tile([128, NK, NT, 128], BF16)
        with tc.tile_pool(name="cqpool", bufs=2) as cqpool:
            for t in range(NT):
                src = ctx_sb[:, t, :, :].rearrange("p c f -> p (c f)")
                amax = cqpool.tile([128, 1], F32, tag="amax_cq")
                nc.vector.tensor_reduce(amax[:], src, AX.X, OP.max,
                                        apply_absolute_value=True)
                cq = cqpool.tile([128, H], BF16, tag="cq")
                _quant_pair(nc, cqpool, src, H, amax[:], sc[:, t:t + 1],
                            dqc[:, t:t + 1], cq[:], magic_ap, "cq")
                nc.sync.dma_start_transpose(ctxqT[:, :, t, :], cq[:])
        ctx_pool.release()

        # ---------- P5: o_proj + residual -> x1_d (+ ffn-norm ssq fused) ----------
        # ---------- P6: ffn rmsnorm (slim) + transpose ----------
        hnT_pool = tc.alloc_tile_pool(name="hnT_pool", bufs=1)
        hnT = hnT_pool.tile([128, NK, NT, 128], BF16)
        with tc.tile_pool(name="opool", bufs=3) as opool, \
             tc.tile_pool(name="owpool", bufs=2) as owpool, \
             tc.tile_pool(name="n2pool", bufs=2) as n2pool, \
             tc.tile_pool(name="n2wpool", bufs=1) as n2wpool, \
             tc.tile_pool(name="o_psum", bufs=3, space="PSUM") as psum_o:
            fnw_b = n2wpool.tile([128, H], F32, tag="normw2")
            ap2 = fnw_in[:]
            nc.gpsimd.dma_start(out=fnw_b[:], in_=bass.AP(
                tensor=ap2.tensor, offset=ap2.offset, ap=[[0, 128]] + list(ap2.ap)))
            for nn in range(4):
                wst = owpool.tile([128, NK, 512], BF16, tag="wo_st")
                nc.gpsimd.dma_start(wst[:], wo_in[nn, :, :, :])
                for t in range(NT):
                    ps = psum_o.tile([128, 512], F32, tag="ps_o")
                    for kk in range(NK):
                        nc.tensor.matmul(ps[:], ctxqT[:, kk, t, :], wst[:, kk, :],
                                         start=(kk == 0), stop=(kk == NK - 1))
                    xs = opool.tile([128, 512], F32, tag="xs")
                    nc.scalar.dma_start(xs[:], x_in[t * 128:(t + 1) * 128,
                                                    nn * 512:(nn + 1) * 512])
                    tmp = opool.tile([128, 512], F32, tag="o_tmp")
                    nc.scalar.activation(tmp[:], ps[:], ACTF.Identity, scale=c_o)
                    x1s = opool.tile([128, 512], F32, tag="x1s")
                    nc.vector.tensor_tensor(x1s[:], tmp[:], xs[:], OP.add)
                    sqt = opool.tile([128, 512], F32, tag="o_sq")
                    nc.vector.tensor_tensor(sqt[:], x1s[:], x1s[:], OP.mult)
                    nc.vector.tensor_reduce(ssq_parts[:, t, nn:nn + 1], sqt[:],
                                            AX.X, OP.add)
                    nc.scalar.dma_start(x1_d[t, :, nn * 512:(nn + 1) * 512], x1s[:])
            for t in range(NT):
                x1_t = n2pool.tile([128, H], F32, tag="x1n")
                nc.scalar.dma_start(x1_t[:], x1_d[t, :, :])
                ssq = n2pool.tile([128, 1], F32, tag="ssq2")
                nc.vector.tensor_reduce(ssq[:], ssq_parts[:, t, :], AX.X, OP.add)
                msq = n2pool.tile([128, 1], F32, tag="msq2")
                nc.vector.tensor_scalar(msq[:], ssq[:], 1.0 / H, EPS, OP.mult, OP.add)
                sd = n2pool.tile([128, 1], F32, tag="sd2")
                nc.scalar.activation(sd[:], msq[:], ACTF.Sqrt)
                rstd = n2pool.tile([128, 1], F32, tag="rstd2")
                nc.vector.reciprocal(rstd[:], sd[:])
                hn_t = n2pool.tile([128, H], BF16, tag="hn_t")
                nc.vector.tensor_scalar_mul(hn_t[:], x1_t[:], rstd[:])
                nc.vector.tensor_tensor(hn_t[:], hn_t[:], fnw_b[:], OP.mult)
                nc.sync.dma_start_transpose(hnT[:, :, t, :], hn_t[:])
        ctxq_pool.release()

        # ---------- P7: ffn up + silu*val -> act_d ----------
        with tc.tile_pool(name="upool", bufs=2) as upool, \
             tc.tile_pool(name="fpool", bufs=3) as fpool, \
             tc.tile_pool(name="up_psum", bufs=2, space="PSUM", side="right") as psum_up, \
             tc.tile_pool(name="upv_psum", bufs=2, space="PSUM", side="right") as psum_upv:
            for i in range(8):   # paired gate/val strips of 512
                wgr = upool.tile([128, NK, 512], BF16, tag="w_up")
                nc.gpsimd.dma_start(wgr[:], wup_in[i, :, :, :])
                wvr = upool.tile([128, NK, 512], BF16, tag="w_up")
                nc.gpsimd.dma_start(wvr[:], wup_in[8 + i, :, :, :])
                for t in range(NT):
                    psg = psum_up.tile([128, 512], F32, tag="ps_g")
                    for kk in range(NK):
                        nc.tensor.matmul(psg[:], hnT[:, kk, t, :], wgr[:, kk, :],
                                         start=(kk == 0), stop=(kk == NK - 1))
                    psv = psum_upv.tile([128, 512], F32, tag="ps_v")
                    for kk in range(NK):
                        nc.tensor.matmul(psv[:], hnT[:, kk, t, :], wvr[:, kk, :],
                                         start=(kk == 0), stop=(kk == NK - 1))
                    sgm = fpool.tile([128, 512], F32, tag="sgm")
                    nc.scalar.activation(sgm[:], psg[:], ACTF.Sigmoid)
                    sg = fpool.tile([128, 512], F32, tag="sg")
                    nc.vector.tensor_tensor(sg[:], sgm[:], psg[:], OP.mult)
                    av = fpool.tile([128, 512], BF16, tag="av")
                    nc.vector.tensor_tensor(av[:], sg[:], psv[:], OP.mult)
                    nc.scalar.dma_start(act_d[t, :, i * 512:(i + 1) * 512], av[:])
                    rmax = fpool.tile([128, 1], F32, tag="rmax")
                    nc.vector.tensor_reduce(rmax[:], av[:], AX.X, OP.max,
                                            apply_absolute_value=True)
                    nc.vector.tensor_tensor(amax_av[:, t:t + 1], amax_av[:, t:t + 1],
                                            rmax[:], OP.max)

        # ---------- P8: ffn act-qdq + transpose -> act2_d ----------
        with tc.tile_pool(name="aqpool", bufs=2) as aqpool:
            for t in range(NT):
                amc = aqpool.tile([128, 1], F32, tag="amc_a")
                nc.vector.tensor_scalar_max(amc[:], amax_av[:, t:t + 1], 1e-5)
                rec = aqpool.tile([128, 1], F32, tag="rec_a")
                nc.vector.reciprocal(rec[:], amc[:])
                nc.vector.tensor_scalar_mul(sa[:, t:t + 1], rec[:], 127.0)
                nc.vector.tensor_scalar_mul(dqa[:, t:t + 1], amc[:], 1.0 / 127.0)
                for hf in range(4):
                    at = aqpool.tile([128, 1024], BF16, tag="at")
                    nc.scalar.dma_start(at[:], act_d[t, :, hf * 1024:(hf + 1) * 1024])
                    mg = aqpool.tile([128, 1024], F32, tag="mg_a")
                    nc.scalar.activation(mg[:], at[:], ACTF.Identity, bias=magic_ap,
                                         scale=sa[:, t:t + 1])
                    aq = aqpool.tile([128, 1024], BF16, tag="aq")
                    nc.vector.tensor_scalar(aq[:], mg[:], float(MAGIC), dqa[:, t:t + 1],
                                            OP.subtract, OP.mult)
                    aqT = aqpool.tile([128, 8, 128], BF16, tag="aqT")
                    nc.sync.dma_start_transpose(aqT[:], aq[:])
                    nc.sync.dma_start(act2_d[t, :, hf * 8:(hf + 1) * 8, :], aqT[:])

        # ---------- P9: ffn down + residual -> out ----------
        with tc.tile_pool(name="dpool", bufs=2) as dpool, \
             tc.tile_pool(name="dspool", bufs=3) as dspool, \
             tc.tile_pool(name="dopool", bufs=3) as dopool, \
             tc.tile_pool(name="dn_psum", bufs=3, space="PSUM") as psum_dn:
            for nn in range(4):
                wsa = dpool.tile([128, 16, 512], BF16, tag="w_dn_a")
                nc.gpsimd.dma_start(wsa[:], wdn_in[nn, 0, :, :, :])
                wsb = dpool.tile([128, 16, 512], BF16, tag="w_dn_b")
                nc.gpsimd.dma_start(wsb[:], wdn_in[nn, 1, :, :, :])
                for t in range(NT):
                    aqt = dspool.tile([128, NKI, 128], BF16, tag="aq_st")
                    nc.gpsimd.dma_start(aqt[:], act2_d[t, :, :, :])
                    ps = psum_dn.tile([128, 512], F32, tag="ps_dn")
                    for kk in range(NKI):
                        w = wsa[:, kk, :] if kk < 16 else wsb[:, kk - 16, :]
                        nc.tensor.matmul(ps[:], aqt[:, kk, :], w,
                                         start=(kk == 0), stop=(kk == NKI - 1))
                    x1_t = dopool.tile([128, 512], F32, tag="x1_re")
                    nc.scalar.dma_start(x1_t[:], x1_d[t, :, nn * 512:(nn + 1) * 512])
                    tmp = dopool.tile([128, 512], F32, tag="d_tmp")
                    nc.scalar.activation(tmp[:], ps[:], ACTF.Identity, scale=c_dn)
                    ot = dopool.tile([128, 512], F32, tag="ot")
                    nc.vector.tensor_tensor(ot[:], tmp[:], x1_t[:], OP.add)
                    nc.scalar.dma_start(out_d[t * 128:(t + 1) * 128,
                                              nn * 512:(nn + 1) * 512], ot[:])
        hnT_pool.release()
        cs_pool.release()
        perm.release()

    nc.compile()
    return nc


_NC_CACHE = None


def _get_nc():
    global _NC_CACHE
    if _NC_CACHE is None:
        _NC_CACHE = build_program()
    return _NC_CACHE


def _ternarize(w):
    """Reference _weight_quant: returns (ternary float {-1,0,1}, dqw scale)."""
    w = np.asarray(w, np.float32)
    m = np.maximum(np.mean(np.abs(w), dtype=np.float32), np.float32(1e-5))
    scale = np.float32(1.0) / m
    tern = np.clip(np.round(w * scale), -1.0, 1.0).astype(np.float32)
    return tern, float(m)


def _strip_layout(w_t, n_strips, nk):
    """[in_feats, out_feats] -> [n_strips, 128, nk, 512] (strip s covers out
    cols s*512..). w_t is the transposed weight [in, out]."""
    infeat, outfeat = w_t.shape
    assert infeat == nk * 128 and outfeat == n_strips * 512
    # [nk, 128, n_strips, 512] -> [n_strips, 128, nk, 512]
    v = w_t.reshape(nk, 128, n_strips, 512)
    return np.ascontiguousarray(v.transpose(2, 1, 0, 3))


def _host_inputs(x, attn_norm_w, ffn_norm_w, qkv_w, o_w, ffn_up_w, ffn_down_w):
    x = np.ascontiguousarray(np.asarray(x, np.float32))
    anw = np.ascontiguousarray(np.asarray(attn_norm_w, np.float32))
    fnw = np.ascontiguousarray(np.asarray(ffn_norm_w, np.float32))

    tern_qkv, dqw_qkv = _ternarize(qkv_w)
    tern_o, dqw_o = _ternarize(o_w)
    tern_dn, dqw_dn = _ternarize(ffn_down_w)

    wqkv_sh = _strip_layout(tern_qkv.T, 12, NK).astype(ml_dtypes.bfloat16)
    wo_sh = _strip_layout(tern_o.T, 4, NK).astype(ml_dtypes.bfloat16)
    wup_sh = _strip_layout(np.asarray(ffn_up_w, np.float32).T, 16, NK) \
        .astype(ml_dtypes.bfloat16)
    wdn_sh = _strip_layout(tern_dn.T, 4, NKI).astype(ml_dtypes.bfloat16) \
        .reshape(4, 128, 2, 16, 512).transpose(0, 2, 1, 3, 4)
    wdn_sh = np.ascontiguousarray(wdn_sh)

    consts = np.zeros(8, np.float32)
    consts[0] = dqw_qkv * dqw_qkv * (HD ** -0.5)
    consts[1] = dqw_qkv * dqw_o
    consts[2] = dqw_dn

    # rope tables: cs[p, 0/1, t, 4*64] bf16, replicated x4 for the 4-head strips
    inv = 1.0 / (THETA ** (np.arange(0, HD, 2, dtype=np.float32) / HD))
    tpos = np.arange(S, dtype=np.float32)
    fr = np.outer(tpos, inv)                     # [S, 64]
    cos = np.tile(np.cos(fr), (1, 4))            # [S, 256]
    sin = np.tile(np.sin(fr), (1, 4))

    in_maps = []
    for c in range(NCORES):
        b = c // 4
        t0 = (c % 4) * R
        # [R, 256] -> [NT, 128, 256] -> [128, NT, 256]
        cs = np.stack([cos[t0:t0 + R].reshape(NT, 128, 256).transpose(1, 0, 2),
                       sin[t0:t0 + R].reshape(NT, 128, 256).transpose(1, 0, 2)],
                      axis=1)                    # [128, 2, NT, 256]
        in_maps.append({
            "x_sh": np.ascontiguousarray(x[b, t0:t0 + R, :]),
            "cs_sh": np.ascontiguousarray(cs).astype(ml_dtypes.bfloat16),
            "attn_norm_w": anw, "ffn_norm_w": fnw, "consts": consts,
            "wqkv_sh": wqkv_sh, "wo_sh": wo_sh, "wup_sh": wup_sh, "wdn_sh": wdn_sh,
        })
    return in_maps


def run(trace=False, **inputs):
    nc = _get_nc()
    in_maps = _host_inputs(**inputs)
    res = run_bass_kernel_spmd(nc, in_maps, list(range(NCORES)), trace=trace)
    out = np.empty((B, S, H), np.float32)
    for c in range(NCORES):
        b = c // 4
        t0 = (c % 4) * R
        out[b, t0:t0 + R, :] = res.results[c]["out_sh"]
    return out, res


def kernel(**inputs):
    out, _ = run(trace=False, **inputs)
    return out


# revision 27
# speedup vs baseline: 1.1453x; 1.1453x over previous
"""Trainium2 Bass kernel for nn_BlockAttentionResidual (block attention + BitNet-style quantized MLP).

Sharding: sequence-block data parallelism, zero collectives. Block attention is
independent per 512-token block, so each of the 8 cores owns 1024 contiguous
tokens (2 blocks) of one batch element and runs the whole layer on them.
  core c -> batch c//4, tokens [(c%4)*1024, (c%4+1)*1024)

Weights are static parameters: ternarization (per-tensor mean|w| scale, exact
reference semantics) is host-side preprocessing; the ternary {-1,0,1} values are
exact in bf16.  Per-tensor dequant scalars fold into three constants shipped as
a tiny input tensor (exp-scale for attention, psum-eviction scales for o_proj /
ffn_down), so the device does no dequant bookkeeping: activations are
quantize-dequantized in one fused ACT+DVE pair per tile and all matmuls run on
bf16 operands with fp32 PSUM accumulation.

All DRAM layouts are pre-tiled host-side so every DMA line is >=1KB contiguous
per partition (the previous kernel was DMA-descriptor-bound: 600k descriptors
averaging 800B).
"""

import numpy as np
import ml_dtypes

import concourse.bass as bass
import concourse.mybir as mybir
import concourse.tile as tile
from concourse import bacc
from concourse.bass_utils import run_bass_kernel_spmd

F32 = mybir.dt.float32
BF16 = mybir.dt.bfloat16
AX = mybir.AxisListType
OP = mybir.AluOpType
ACTF = mybir.ActivationFunctionType

# model dims
H = 2048
NH = 16
HD = 128
NB = 8
INTER = 4096        # 2*H
EPS = 1e-5
THETA = 10000.0
B, S = 2, 4096
BT = 512            # tokens per attention block
NCORES = 8
R = 1024            # tokens per core
NT = R // 128       # 8 token tiles per core
NK = H // 128       # 16 k-tiles of the hidden dim
NKI = INTER // 128  # 32 k-tiles of the intermediate dim
MAGIC = np.float32(1.5 * 2 ** 23)   # fp32 round-to-nearest-even magic


def _quant_pair(nc, pool, src_ap, ncols, amax_ap, s_store, dq_store, out_bf,
                magic_ap, tag):
    """Quantize-dequantize src_ap [128, ncols] onto the int8 grid:
    out_bf = round(src*127/amax) * amax/127 in bf16.  amax_ap: [128,1] f32."""
    amc = pool.tile([128, 1], F32, tag=f"amc_{tag}")
    nc.vector.tensor_scalar_max(amc[:], amax_ap, 1e-5)
    rec = pool.tile([128, 1], F32, tag=f"rec_{tag}")
    nc.vector.reciprocal(rec[:], amc[:])
    nc.vector.tensor_scalar_mul(s_store, rec[:], 127.0)
    nc.vector.tensor_scalar_mul(dq_store, amc[:], 1.0 / 127.0)
    mg = pool.tile([128, ncols], F32, tag=f"mg_{tag}")
    nc.scalar.activation(mg[:], src_ap, ACTF.Identity, bias=magic_ap, scale=s_store)
    nc.vector.tensor_scalar(out_bf, mg[:], float(MAGIC), dq_store, OP.subtract,
                            OP.mult)


def build_program():
    nc = bacc.Bacc(None, target_bir_lowering=False)

    # ---- I/O (all layouts pre-tiled on host) ----
    x_in = nc.declare_dram_parameter("x_sh", [R, H], F32, isOutput=False)
    cs_in = nc.declare_dram_parameter("cs_sh", [128, 2, NT, 256], BF16, isOutput=False)
    anw_in = nc.declare_dram_parameter("attn_norm_w", [H], F32, isOutput=False)
    fnw_in = nc.declare_dram_parameter("ffn_norm_w", [H], F32, isOutput=False)
    consts_in = nc.declare_dram_parameter("consts", [8], F32, isOutput=False)
    # weight strips: [strip, 128 kpart, n_ktiles, 512 outcols] bf16
    wqkv_in = nc.declare_dram_parameter("wqkv_sh", [12, 128, NK, 512], BF16, isOutput=False)
    wo_in = nc.declare_dram_parameter("wo_sh", [4, 128, NK, 512], BF16, isOutput=False)
    wup_in = nc.declare_dram_parameter("wup_sh", [16, 128, NK, 512], BF16, isOutput=False)
    # ffn_down strips split in two k-halves: [strip, half, 128, 16, 512]
    wdn_in = nc.declare_dram_parameter("wdn_sh", [4, 2, 128, 16, 512], BF16, isOutput=False)
    out_d = nc.declare_dram_parameter("out_sh", [R, H], F32, isOutput=True)

    # ---- internal DRAM scratch ----
    x1_d = nc.dram_tensor("x1_d", [NT, 128, H], F32)
    act_d = nc.dram_tensor("act_d", [NT, 128, INTER], BF16)      # silu(g)*v rows
    act2_d = nc.dram_tensor("act2_d", [NT, 128, NKI, 128], BF16)  # quantized, transposed

    with tile.TileContext(nc) as tc:
        perm = tc.alloc_tile_pool(name="perm", bufs=1)
        magic_t = perm.tile([128, 1], F32)
        nc.vector.memset(magic_t[:], float(MAGIC))
        magic_ap = magic_t[:]
        consts_b = perm.tile([128, 8], F32)
        ap0 = consts_in[:]
        nc.gpsimd.dma_start(out=consts_b[:], in_=bass.AP(
            tensor=ap0.tensor, offset=ap0.offset, ap=[[0, 128]] + list(ap0.ap)))
        c_att = consts_b[:, 0:1]   # dqw_qkv^2 * HD^-0.5
        c_o = consts_b[:, 1:2]     # dqw_qkv * dqw_o
        c_dn = consts_b[:, 2:3]    # dqw_dn
        # per-token quant scales (s = 127/amax, dq = amax/127)
        s1 = perm.tile([128, NT], F32)
        dq1 = perm.tile([128, NT], F32)
        sc = perm.tile([128, NT], F32)
        dqc = perm.tile([128, NT], F32)
        sa = perm.tile([128, NT], F32)
        dqa = perm.tile([128, NT], F32)
        amax_av = perm.tile([128, NT], F32)
        nc.vector.memset(amax_av[:], 0.0)

        ssq_parts = perm.tile([128, NT, 4], F32)

        cs_pool = tc.alloc_tile_pool(name="cs_pool", bufs=1)
        cs_b = cs_pool.tile([128, 2, NT, 256], BF16)
        nc.gpsimd.dma_start(cs_b[:], cs_in[:])

        ctx_pool = tc.alloc_tile_pool(name="ctx_pool", bufs=1)
        ctx_sb = ctx_pool.tile([128, NT, NH, 128], BF16)
        slot1 = tc.alloc_tile_pool(name="slot1", bufs=1)
        xqT = slot1.tile([128, NK, NT, 128], BF16)

        # ---------- P1: attn rmsnorm + act-qdq + transpose ----------
        with tc.tile_pool(name="npool", bufs=2) as npool, \
             tc.tile_pool(name="nwpool", bufs=1) as nwpool:
            anw_b = nwpool.tile([128, H], F32, tag="normw")
            ap1 = anw_in[:]
            nc.gpsimd.dma_start(out=anw_b[:], in_=bass.AP(
                tensor=ap1.tensor, offset=ap1.offset, ap=[[0, 128]] + list(ap1.ap)))
            for t in range(NT):
                xt = npool.tile([128, H], F32, tag="xt")
                nc.scalar.dma_start(xt[:], x_in[t * 128:(t + 1) * 128, :])
                ssq = npool.tile([128, 1], F32, tag="ssq")
                junk = npool.tile([128, H], BF16, tag="njunk")
                nc.scalar.activation(junk[:], xt[:], ACTF.Square, accum_out=ssq[:])
                msq = npool.tile([128, 1], F32, tag="msq")
                nc.vector.tensor_scalar(msq[:], ssq[:], 1.0 / H, EPS, OP.mult, OP.add)
                sd = npool.tile([128, 1], F32, tag="sd")
                nc.scalar.activation(sd[:], msq[:], ACTF.Sqrt)
                rstd = npool.tile([128, 1], F32, tag="rstd")
                nc.vector.reciprocal(rstd[:], sd[:])
                h_t = npool.tile([128, H], F32, tag="h_t")
                nc.vector.tensor_scalar_mul(h_t[:], xt[:], rstd[:])
                nc.vector.tensor_tensor(h_t[:], h_t[:], anw_b[:], OP.mult)
                amax = npool.tile([128, 1], F32, tag="amax_n1")
                nc.vector.tensor_reduce(amax[:], h_t[:], AX.X, OP.max,
                                        apply_absolute_value=True)
                xq = npool.tile([128, H], BF16, tag="xq")
                _quant_pair(nc, npool, h_t[:], H, amax[:], s1[:, t:t + 1],
                            dq1[:, t:t + 1], xq[:], magic_ap, "n1")
                nc.sync.dma_start_transpose(xqT[:, :, t, :], xq[:])

        # ---------- P2+P3: qkv matmul + rope interleaved with block attention ----------
        # strips ordered q_g,k_g,v_g per 4-head group g; attention for group g
        # runs right after its three strips, overlapping later groups' matmuls.
        with tc.tile_pool(name="qkwpool", bufs=2) as qkwpool, \
             tc.tile_pool(name="ropool", bufs=3) as ropool, \
             tc.tile_pool(name="qkpool", bufs=2) as qkpool, \
             tc.tile_pool(name="apool", bufs=2) as apool, \
             tc.tile_pool(name="qkv_psum", bufs=3, space="PSUM") as psum_mm, \
             tc.tile_pool(name="at_psum", bufs=3, space="PSUM", side="right") as psum_at, \
             tc.tile_pool(name="ctx_psum", bufs=2, space="PSUM", side="right") as psum_cx:
            for g in range(4):
                qTg = qkpool.tile([128, 4, NT, 128], BF16, tag="qTg")
                kTg = qkpool.tile([128, 4, NT, 128], BF16, tag="kTg")
                v_g = qkpool.tile([128, NT, 4, 132], BF16, tag="vg")
                nc.vector.memset(v_g[:, :, :, 128:129], 1.0)
                for kind in range(3):
                    src = g + 4 * kind
                    wst = qkwpool.tile([128, NK, 512], BF16, tag="w_qkv")
                    nc.gpsimd.dma_start(wst[:], wqkv_in[src, :, :, :])
                    for t in range(NT):
                        ps = psum_mm.tile([128, 512], F32, tag="ps_qkv")
                        for kk in range(NK):
                            nc.tensor.matmul(ps[:], xqT[:, kk, t, :], wst[:, kk, :],
                                             start=(kk == 0), stop=(kk == NK - 1))
                        if kind == 2:
                            nc.scalar.copy(v_g[:, t, :, 0:128],
                                           ps[:].rearrange("p (c f) -> p c f", c=4))
                        else:
                            qsc = ropool.tile([128, 4, 128], BF16, tag="qsc")
                            nc.scalar.copy(qsc[:], ps[:].rearrange("p (c f) -> p c f", c=4))
                            p1, p2 = qsc[:, :, 0:64], qsc[:, :, 64:128]
                            cosd = cs_b[:, 0, t, :].rearrange("p (c f) -> p c f", c=4)
                            sind = cs_b[:, 1, t, :].rearrange("p (c f) -> p c f", c=4)
                            t1 = ropool.tile([128, 4, 64], F32, tag="rt1")
                            t2 = ropool.tile([128, 4, 64], F32, tag="rt2")
                            rot = ropool.tile([128, 4, 128], BF16, tag="rot")
                            nc.vector.tensor_tensor(t1[:], p1, cosd, OP.mult)
                            nc.gpsimd.tensor_tensor(t2[:], p2, sind, OP.mult)
                            nc.vector.tensor_tensor(rot[:, :, 0:64], t1[:], t2[:], OP.subtract)
                            nc.vector.tensor_tensor(t1[:], p2, cosd, OP.mult)
                            nc.gpsimd.tensor_tensor(t2[:], p1, sind, OP.mult)
                            nc.vector.tensor_tensor(rot[:, :, 64:128], t1[:], t2[:], OP.add)
                            dst = qTg if kind == 0 else kTg
                            nc.sync.dma_start_transpose(
                                dst[:, :, t, :], rot[:].rearrange("p c f -> p (c f)"))
                for blk in range(2):
                    for hl in range(4):
                        expT = [None] * 4
                        for kt in range(4):
                            qn = 512 - kt * 128
                            pss = psum_at.tile([128, 512], F32, tag="ps_sc")
                            nc.tensor.matmul(
                                pss[:, 0:qn],
                                kTg[:, hl, blk * 4 + kt, :],
                                qTg[:, hl, blk * 4:(blk + 1) * 4, :]
                                .rearrange("p c f -> p (c f)")[:, kt * 128:512],
                                start=True, stop=True)
                            ex = apool.tile([128, 512], BF16, tag=f"expT{kt}")
                            nc.scalar.activation(ex[:, 0:qn], pss[:, 0:qn], ACTF.Exp,
                                                 scale=c_att)
                            nc.gpsimd.affine_select(
                                out=ex[:, 0:128], in_=ex[:, 0:128],
                                compare_op=OP.is_ge, fill=0.0,
                                base=0, pattern=[[1, 128]], channel_multiplier=-1)
                            expT[kt] = ex
                        for qt in range(4):
                            psc = psum_cx.tile([128, 132], F32, tag="ps_ctx")
                            for kt in range(qt + 1):
                                nc.tensor.matmul(psc[:, 0:129],
                                                 expT[kt][:, (qt - kt) * 128:(qt - kt) * 128 + 128],
                                                 v_g[:, blk * 4 + kt, hl, 0:129],
                                                 start=(kt == 0), stop=(kt == qt))
                            rl = apool.tile([128, 1], F32, tag="rl")
                            nc.vector.reciprocal(rl[:], psc[:, 128:129])
                            nc.vector.tensor_scalar_mul(
                                ctx_sb[:, blk * 4 + qt, 4 * g + hl, :],
                                psc[:, 0:128], rl[:])
        slot1.release()

        # ---------- P4: ctx act-qdq + transpose ----------
        ctxq_pool = tc.alloc_tile_pool(name="ctxq_pool", bufs=1, side="right")
        ctxqT = ctxq_pool.tile([128, NK, NT, 128], BF16)
        with tc.tile_pool(name="cqpool", bufs=2) as cqpool:
            for t in range(NT):
                src = ctx_sb[:, t, :, :].rearrange("p c f -> p (c f)")
                amax = cqpool.tile([128, 1], F32, tag="amax_cq")
                nc.vector.tensor_reduce(amax[:], src, AX.X, OP.max,
                                        apply_absolute_value=True)
                cq = cqpool.tile([128, H], BF16, tag="cq")
                _quant_pair(nc, cqpool, src, H, amax[:], sc[:, t:t + 1],
                            dqc[:, t:t + 1], cq[:], magic_ap, "cq")
                nc.sync.dma_start_transpose(ctxqT[:, :, t, :], cq[:])
        ctx_pool.release()

        # ---------- P5: o_proj + residual -> x1_d (+ ffn-norm ssq fused) ----------
        # ---------- P6: ffn rmsnorm (slim) + transpose ----------
        hnT_pool = tc.alloc_tile_pool(name="hnT_pool", bufs=1)
        hnT = hnT_pool.tile([128, NK, NT, 128], BF16)
        with tc.tile_pool(name="opool", bufs=3) as opool, \
             tc.tile_pool(name="owpool", bufs=2) as owpool, \
             tc.tile_pool(name="n2pool", bufs=2) as n2pool, \
             tc.tile_pool(name="n2wpool", bufs=1) as n2wpool, \
             tc.tile_pool(name="o_psum", bufs=3, space="PSUM") as psum_o:
            fnw_b = n2wpool.tile([128, H], F32, tag="normw2")
            ap2 = fnw_in[:]
            nc.gpsimd.dma_start(out=fnw_b[:], in_=bass.AP(
                tensor=ap2.tensor, offset=ap2.offset, ap=[[0, 128]] + list(ap2.ap)))
            for nn in range(4):
                wst = owpool.tile([128, NK, 512], BF16, tag="wo_st")
                nc.gpsimd.dma_start(wst[:], wo_in[nn, :, :, :])
                for t in range(NT):
                    ps = psum_o.tile([128, 512], F32, tag="ps_o")
                    for kk in range(NK):
                        nc.tensor.matmul(ps[:], ctxqT[:, kk, t, :], wst[:, kk, :],
                                         start=(kk == 0), stop=(kk == NK - 1))
                    xs = opool.tile([128, 512], F32, tag="xs")
                    nc.scalar.dma_start(xs[:], x_in[t * 128:(t + 1) * 128,
                                                    nn * 512:(nn + 1) * 512])
                    tmp = opool.tile([128, 512], F32, tag="o_tmp")
                    nc.scalar.activation(tmp[:], ps[:], ACTF.Identity, scale=c_o)
                    x1s = opool.tile([128, 512], F32, tag="x1s")
                    nc.vector.tensor_tensor(x1s[:], tmp[:], xs[:], OP.add)
                    sqt = opool.tile([128, 512], F32, tag="o_sq")
                    nc.vector.tensor_tensor(sqt[:], x1s[:], x1s[:], OP.mult)
                    nc.vector.tensor_reduce(ssq_parts[:, t, nn:nn + 1], sqt[:],
                                            AX.X, OP.add)
                    nc.scalar.dma_start(x1_d[t, :, nn * 512:(nn + 1) * 512], x1s[:])
            for t in range(NT):
                x1_t = n2pool.tile([128, H], F32, tag="x1n")
                nc.scalar.dma_start(x1_t[:], x1_d[t, :, :])
                ssq = n2pool.tile([128, 1], F32, tag="ssq2")
                nc.vector.tensor_reduce(ssq[:], ssq_parts[:, t, :], AX.X, OP.add)
                msq = n2pool.tile([128, 1], F32, tag="msq2")
                nc.vector.tensor_scalar(msq[:], ssq[:], 1.0 / H, EPS, OP.mult, OP.add)
                sd = n2pool.tile([128, 1], F32, tag="sd2")
                nc.scalar.activation(sd[:], msq[:], ACTF.Sqrt)
                rstd = n2pool.tile([128, 1], F32, tag="rstd2")
                nc.vector.reciprocal(rstd[:], sd[:])
                hn_t = n2pool.tile([128, H], BF16, tag="hn_t")
                nc.vector.tensor_scalar_mul(hn_t[:], x1_t[:], rstd[:])
                nc.vector.tensor_tensor(hn_t[:], hn_t[:], fnw_b[:], OP.mult)
                nc.sync.dma_start_transpose(hnT[:, :, t, :], hn_t[:])
        ctxq_pool.release()

        # ---------- P7: ffn up + silu*val -> act_d ----------
        with tc.tile_pool(name="upool", bufs=2) as upool, \
             tc.tile_pool(name="fpool", bufs=3) as fpool, \
             tc.tile_pool(name="up_psum", bufs=2, space="PSUM", side="right") as psum_up, \
             tc.tile_pool(name="upv_psum", bufs=2, space="PSUM", side="right") as psum_upv:
            for i in range(8):   # paired gate/val strips of 512
                wgr = upool.tile([128, NK, 512], BF16, tag="w_up")
                nc.gpsimd.dma_start(wgr[:], wup_in[i, :, :, :])
                wvr = upool.tile([128, NK, 512], BF16, tag="w_up")
                nc.gpsimd.dma_start(wvr[:], wup_in[8 + i, :, :, :])
                for t in range(NT):
                    psg = psum_up.tile([128, 512], F32, tag="ps_g")
                    for kk in range(NK):
                        nc.tensor.matmul(psg[:], hnT[:, kk, t, :], wgr[:, kk, :],
                                         start=(kk == 0), stop=(kk == NK - 1))
                    psv = psum_upv.tile([128, 512], F32, tag="ps_v")
                    for kk in range(NK):
                        nc.tensor.matmul(psv[:], hnT[:, kk, t, :], wvr[:, kk, :],
                                         start=(kk == 0), stop=(kk == NK - 1))
                    sgm = fpool.tile([128, 512], F32, tag="sgm")
                    nc.scalar.activation(sgm[:], psg[:], ACTF.Sigmoid)
                    sg = fpool.tile([128, 512], F32, tag="sg")
                    nc.vector.tensor_tensor(sg[:], sgm[:], psg[:], OP.mult)
                    av = fpool.tile([128, 512], BF16, tag="av")
                    nc.vector.tensor_tensor(av[:], sg[:], psv[:], OP.mult)
                    nc.sync.dma_start(act_d[t, :, i * 512:(i + 1) * 512], av[:])
                    rmax = fpool.tile([128, 1], F32, tag="rmax")
                    nc.vector.tensor_reduce(rmax[:], av[:], AX.X, OP.max,
                                            apply_absolute_value=True)
                    nc.vector.tensor_tensor(amax_av[:, t:t + 1], amax_av[:, t:t + 1],
                                            rmax[:], OP.max)

        # ---------- P8: ffn act-qdq + transpose -> act2_d ----------
        with tc.tile_pool(name="aqpool", bufs=2) as aqpool:
            for t in range(NT):
                amc = aqpool.tile([128, 1], F32, tag="amc_a")
                nc.vector.tensor_scalar_max(amc[:], amax_av[:, t:t + 1], 1e-5)
                rec = aqpool.tile([128, 1], F32, tag="rec_a")
                nc.vector.reciprocal(rec[:], amc[:])
                nc.vector.tensor_scalar_mul(sa[:, t:t + 1], rec[:], 127.0)
                nc.vector.tensor_scalar_mul(dqa[:, t:t + 1], amc[:], 1.0 / 127.0)
                for hf in range(4):
                    at = aqpool.tile([128, 1024], BF16, tag="at")
                    nc.gpsimd.dma_start(at[:], act_d[t, :, hf * 1024:(hf + 1) * 1024])
                    mg = aqpool.tile([128, 1024], F32, tag="mg_a")
                    nc.scalar.activation(mg[:], at[:], ACTF.Identity, bias=magic_ap,
                                         scale=sa[:, t:t + 1])
                    aq = aqpool.tile([128, 1024], BF16, tag="aq")
                    nc.vector.tensor_scalar(aq[:], mg[:], float(MAGIC), dqa[:, t:t + 1],
                                            OP.subtract, OP.mult)
                    aqT = aqpool.tile([128, 8, 128], BF16, tag="aqT")
                    dmae = nc.sync if hf % 2 == 0 else nc.scalar
                    dmae.dma_start_transpose(aqT[:], aq[:])
                    dmae.dma_start(act2_d[t, :, hf * 8:(hf + 1) * 8, :], aqT[:])

        # ---------- P9: ffn down + residual -> out ----------
        with tc.tile_pool(name="dpool", bufs=2) as dpool, \
             tc.tile_pool(name="dspool", bufs=3) as dspool, \
             tc.tile_pool(name="dopool", bufs=3) as dopool, \
             tc.tile_pool(name="dn_psum", bufs=4, space="PSUM") as psum_dn:
            for nn in range(4):
                wsa = dpool.tile([128, 16, 512], BF16, tag="w_dn_a")
                nc.gpsimd.dma_start(wsa[:], wdn_in[nn, 0, :, :, :])
                wsb = dpool.tile([128, 16, 512], BF16, tag="w_dn_b")
                nc.gpsimd.dma_start(wsb[:], wdn_in[nn, 1, :, :, :])
                for t in range(NT):
                    aqt = dspool.tile([128, NKI, 128], BF16, tag="aq_st")
                    nc.gpsimd.dma_start(aqt[:], act2_d[t, :, :, :])
                    ps = psum_dn.tile([128, 512], F32, tag="ps_dn")
                    for kk in range(NKI):
                        w = wsa[:, kk, :] if kk < 16 else wsb[:, kk - 16, :]
                        nc.tensor.matmul(ps[:], aqt[:, kk, :], w,
                                         start=(kk == 0), stop=(kk == NKI - 1))
                    x1_t = dopool.tile([128, 512], F32, tag="x1_re")
                    nc.scalar.dma_start(x1_t[:], x1_d[t, :, nn * 512:(nn + 1) * 512])
                    tmp = dopool.tile([128, 512], F32, tag="d_tmp")
                    nc.scalar.activation(tmp[:], ps[:], ACTF.Identity, scale=c_dn)
                    ot = dopool.tile([128, 512], F32, tag="ot")
                    nc.vector.tensor_tensor(ot[:], tmp[:], x1_t[:], OP.add)
                    nc.scalar.dma_start(out_d[t * 128:(t + 1) * 128,
                                              nn * 512:(nn + 1) * 512], ot[:])
        hnT_pool.release()
        cs_pool.release()
        perm.release()

    nc.compile()
    return nc


_NC_CACHE = None


def _get_nc():
    global _NC_CACHE
    if _NC_CACHE is None:
        _NC_CACHE = build_program()
    return _NC_CACHE


def _ternarize(w):
    """Reference _weight_quant: returns (ternary float {-1,0,1}, dqw scale)."""
    w = np.asarray(w, np.float32)
    m = np.maximum(np.mean(np.abs(w), dtype=np.float32), np.float32(1e-5))
    scale = np.float32(1.0) / m
    tern = np.clip(np.round(w * scale), -1.0, 1.0).astype(np.float32)
    return tern, float(m)


def _strip_layout(w_t, n_strips, nk):
    """[in_feats, out_feats] -> [n_strips, 128, nk, 512] (strip s covers out
    cols s*512..). w_t is the transposed weight [in, out]."""
    infeat, outfeat = w_t.shape
    assert infeat == nk * 128 and outfeat == n_strips * 512
    # [nk, 128, n_strips, 512] -> [n_strips, 128, nk, 512]
    v = w_t.reshape(nk, 128, n_strips, 512)
    return np.ascontiguousarray(v.transpose(2, 1, 0, 3))


def _host_inputs(x, attn_norm_w, ffn_norm_w, qkv_w, o_w, ffn_up_w, ffn_down_w):
    x = np.ascontiguousarray(np.asarray(x, np.float32))
    anw = np.ascontiguousarray(np.asarray(attn_norm_w, np.float32))
    fnw = np.ascontiguousarray(np.asarray(ffn_norm_w, np.float32))

    tern_qkv, dqw_qkv = _ternarize(qkv_w)
    tern_o, dqw_o = _ternarize(o_w)
    tern_dn, dqw_dn = _ternarize(ffn_down_w)

    wqkv_sh = _strip_layout(tern_qkv.T, 12, NK).astype(ml_dtypes.bfloat16)
    wo_sh = _strip_layout(tern_o.T, 4, NK).astype(ml_dtypes.bfloat16)
    wup_sh = _strip_layout(np.asarray(ffn_up_w, np.float32).T, 16, NK) \
        .astype(ml_dtypes.bfloat16)
    wdn_sh = _strip_layout(tern_dn.T, 4, NKI).astype(ml_dtypes.bfloat16) \
        .reshape(4, 128, 2, 16, 512).transpose(0, 2, 1, 3, 4)
    wdn_sh = np.ascontiguousarray(wdn_sh)

    consts = np.zeros(8, np.float32)
    consts[0] = dqw_qkv * dqw_qkv * (HD ** -0.5)
    consts[1] = dqw_qkv * dqw_o
    consts[2] = dqw_dn

    # rope tables: cs[p, 0/1, t, 4*64] bf16, replicated x4 for the 4-head strips
    inv = 1.0 / (THETA ** (np.arange(0, HD, 2, dtype=np.float32) / HD))
    tpos = np.arange(S, dtype=np.float32)
    fr = np.outer(tpos, inv)                     # [S, 64]
    cos = np.tile(np.cos(fr), (1, 4))            # [S, 256]
    sin = np.tile(np.sin(fr), (1, 4))

    in_maps = []
    for c in range(NCORES):
        b = c // 4
        t0 = (c % 4) * R
        # [R, 256] -> [NT, 128, 256] -> [128, NT, 256]
        cs = np.stack([cos[t0:t0 + R].reshape(NT, 128, 256).transpose(1, 0, 2),
                       sin[t0:t0 + R].reshape(NT, 128, 256).transpose(1, 0, 2)],
                      axis=1)                    # [128, 2, NT, 256]
        in_maps.append({
            "x_sh": np.ascontiguousarray(x[b, t0:t0 + R, :]),
            "cs_sh": np.ascontiguousarray(cs).astype(ml_dtypes.bfloat16),
            "attn_norm_w": anw, "ffn_norm_w": fnw, "consts": consts,
            "wqkv_sh": wqkv_sh, "wo_sh": wo_sh, "wup_sh": wup_sh, "wdn_sh": wdn_sh,
        })
    return in_maps


def run(trace=False, **inputs):
    nc = _get_nc()
    in_maps = _host_inputs(**inputs)
    res = run_bass_kernel_spmd(nc, in_maps, list(range(NCORES)), trace=trace)
    out = np.empty((B, S, H), np.float32)
    for c in range(NCORES):
        b = c // 4
        t0 = (c % 4) * R
        out[b, t0:t0 + R, :] = res.results[c]["out_sh"]
    return out, res


def kernel(**inputs):
    out, _ = run(trace=False, **inputs)
    return out


# revision 41
# speedup vs baseline: 1.1786x; 1.0291x over previous
"""Trainium2 Bass kernel for nn_BlockAttentionResidual (block attention + BitNet-style quantized MLP).

Sharding: sequence-block data parallelism, zero collectives. Block attention is
independent per 512-token block, so each of the 8 cores owns 1024 contiguous
tokens (2 blocks) of one batch element and runs the whole layer on them.
  core c -> batch c//4, tokens [(c%4)*1024, (c%4+1)*1024)

Weights are static parameters: ternarization (per-tensor mean|w| scale, exact
reference semantics) is host-side preprocessing; the ternary {-1,0,1} values are
exact in bf16.  Per-tensor dequant scalars fold into three constants shipped as
a tiny input tensor (exp-scale for attention, psum-eviction scales for o_proj /
ffn_down), so the device does no dequant bookkeeping: activations are
quantize-dequantized in one fused ACT+DVE pair per tile and all matmuls run on
bf16 operands with fp32 PSUM accumulation.

All DRAM layouts are pre-tiled host-side so every DMA line is >=1KB contiguous
per partition (the previous kernel was DMA-descriptor-bound: 600k descriptors
averaging 800B).
"""

import numpy as np
import ml_dtypes

import concourse.bass as bass
import concourse.mybir as mybir
import concourse.tile as tile
from concourse import bacc
from concourse.bass_utils import run_bass_kernel_spmd

F32 = mybir.dt.float32
BF16 = mybir.dt.bfloat16
AX = mybir.AxisListType
OP = mybir.AluOpType
ACTF = mybir.ActivationFunctionType

# model dims
H = 2048
NH = 16
HD = 128
NB = 8
INTER = 4096        # 2*H
EPS = 1e-5
THETA = 10000.0
B, S = 2, 4096
BT = 512            # tokens per attention block
NCORES = 8
R = 1024            # tokens per core
NT = R // 128       # 8 token tiles per core
NK = H // 128       # 16 k-tiles of the hidden dim
NKI = INTER // 128  # 32 k-tiles of the intermediate dim
MAGIC = np.float32(1.5 * 2 ** 23)   # fp32 round-to-nearest-even magic


def _quant_pair(nc, pool, src_ap, ncols, amax_ap, s_store, dq_store, out_bf,
                magic_ap, tag):
    """Quantize-dequantize src_ap [128, ncols] onto the int8 grid:
    out_bf = round(src*127/amax) * amax/127 in bf16.  amax_ap: [128,1] f32."""
    amc = pool.tile([128, 1], F32, tag=f"amc_{tag}")
    nc.vector.tensor_scalar_max(amc[:], amax_ap, 1e-5)
    rec = pool.tile([128, 1], F32, tag=f"rec_{tag}")
    nc.vector.reciprocal(rec[:], amc[:])
    nc.vector.tensor_scalar_mul(s_store, rec[:], 127.0)
    nc.vector.tensor_scalar_mul(dq_store, amc[:], 1.0 / 127.0)
    mg = pool.tile([128, ncols], F32, tag=f"mg_{tag}")
    nc.scalar.activation(mg[:], src_ap, ACTF.Identity, bias=magic_ap, scale=s_store)
    nc.vector.tensor_scalar(out_bf, mg[:], float(MAGIC), dq_store, OP.subtract,
                            OP.mult)


def build_program():
    nc = bacc.Bacc(None, target_bir_lowering=False)

    # ---- I/O (all layouts pre-tiled on host) ----
    x_in = nc.declare_dram_parameter("x_sh", [R, H], F32, isOutput=False)
    cs_in = nc.declare_dram_parameter("cs_sh", [128, 2, NT, 256], BF16, isOutput=False)
    anw_in = nc.declare_dram_parameter("attn_norm_w", [H], F32, isOutput=False)
    fnw_in = nc.declare_dram_parameter("ffn_norm_w", [H], F32, isOutput=False)
    consts_in = nc.declare_dram_parameter("consts", [8], F32, isOutput=False)
    # weight strips: [strip, 128 kpart, n_ktiles, 512 outcols] bf16
    wqkv_in = nc.declare_dram_parameter("wqkv_sh", [12, 128, NK, 512], BF16, isOutput=False)
    wo_in = nc.declare_dram_parameter("wo_sh", [4, 128, NK, 512], BF16, isOutput=False)
    wup_in = nc.declare_dram_parameter("wup_sh", [16, 128, NK, 512], BF16, isOutput=False)
    # ffn_down strips split in two k-halves: [strip, half, 128, 16, 512]
    wdn_in = nc.declare_dram_parameter("wdn_sh", [4, 2, 128, 16, 512], BF16, isOutput=False)
    out_d = nc.declare_dram_parameter("out_sh", [R, H], F32, isOutput=True)

    # ---- internal DRAM scratch ----
    x1_d = nc.dram_tensor("x1_d", [NT, 128, H], F32)
    act_d = nc.dram_tensor("act_d", [NT, 128, INTER], BF16)      # silu(g)*v rows

    with tile.TileContext(nc) as tc:
        perm = tc.alloc_tile_pool(name="perm", bufs=1)
        magic_t = perm.tile([128, 1], F32)
        nc.vector.memset(magic_t[:], float(MAGIC))
        magic_ap = magic_t[:]
        consts_b = perm.tile([128, 8], F32)
        ap0 = consts_in[:]
        nc.gpsimd.dma_start(out=consts_b[:], in_=bass.AP(
            tensor=ap0.tensor, offset=ap0.offset, ap=[[0, 128]] + list(ap0.ap)))
        c_att = consts_b[:, 0:1]   # dqw_qkv^2 * HD^-0.5
        c_o = consts_b[:, 1:2]     # dqw_qkv * dqw_o
        c_dn = consts_b[:, 2:3]    # dqw_dn
        # per-token quant scales (s = 127/amax, dq = amax/127)
        s1 = perm.tile([128, NT], F32)
        dq1 = perm.tile([128, NT], F32)
        sc = perm.tile([128, NT], F32)
        dqc = perm.tile([128, NT], F32)
        sa = perm.tile([128, NT], F32)
        dqa = perm.tile([128, NT], F32)
        amax_av = perm.tile([128, NT], F32)
        nc.vector.memset(amax_av[:], 0.0)

        ssq_parts = perm.tile([128, NT, 4], F32)

        cs_pool = tc.alloc_tile_pool(name="cs_pool", bufs=1)
        cs_b = cs_pool.tile([128, 2, NT, 256], BF16)
        nc.gpsimd.dma_start(cs_b[:], cs_in[:])

        ctx_pool = tc.alloc_tile_pool(name="ctx_pool", bufs=1)
        ctx_sb = ctx_pool.tile([128, NT, NH, 128], BF16)
        slot1 = tc.alloc_tile_pool(name="slot1", bufs=1)
        xqT = slot1.tile([128, NT, NK, 128], BF16)

        # qkv weight pool opened early so the first strips prefetch during P1
        qkwpool = tc.alloc_tile_pool(name="qkwpool", bufs=2)
        wst_pre = []
        for kind in range(2):
            wst = qkwpool.tile([128, NK, 512], BF16, tag="w_qkv")
            nc.gpsimd.dma_start(wst[:], wqkv_in[4 * kind, :, :, :])
            wst_pre.append(wst)

        # ---------- P1: attn rmsnorm + act-qdq + transpose ----------
        with tc.tile_pool(name="npool", bufs=2) as npool, \
             tc.tile_pool(name="nwpool", bufs=1) as nwpool:
            anw_b = nwpool.tile([128, H], F32, tag="normw")
            ap1 = anw_in[:]
            nc.gpsimd.dma_start(out=anw_b[:], in_=bass.AP(
                tensor=ap1.tensor, offset=ap1.offset, ap=[[0, 128]] + list(ap1.ap)))
            for t in range(NT):
                xt = npool.tile([128, H], F32, tag="xt")
                nc.scalar.dma_start(xt[:], x_in[t * 128:(t + 1) * 128, :])
                ssq = npool.tile([128, 1], F32, tag="ssq")
                junk = npool.tile([128, H], BF16, tag="njunk")
                nc.scalar.activation(junk[:], xt[:], ACTF.Square, accum_out=ssq[:])
                msq = npool.tile([128, 1], F32, tag="msq")
                nc.vector.tensor_scalar(msq[:], ssq[:], 1.0 / H, EPS, OP.mult, OP.add)
                sd = npool.tile([128, 1], F32, tag="sd")
                nc.scalar.activation(sd[:], msq[:], ACTF.Sqrt)
                rstd = npool.tile([128, 1], F32, tag="rstd")
                nc.vector.reciprocal(rstd[:], sd[:])
                h_t = npool.tile([128, H], F32, tag="h_t")
                nc.vector.tensor_scalar_mul(h_t[:], xt[:], rstd[:])
                nc.gpsimd.tensor_tensor(h_t[:], h_t[:], anw_b[:], OP.mult)
                amax = npool.tile([128, 1], F32, tag="amax_n1")
                nc.vector.tensor_reduce(amax[:], h_t[:], AX.X, OP.max,
                                        apply_absolute_value=True)
                xq = npool.tile([128, H], BF16, tag="xq")
                _quant_pair(nc, npool, h_t[:], H, amax[:], s1[:, t:t + 1],
                            dq1[:, t:t + 1], xq[:], magic_ap, "n1")
                nc.sync.dma_start_transpose(xqT[:, t, :, :], xq[:])

        # ---------- P2+P3: qkv matmul + rope interleaved with block attention ----------
        # strips ordered q_g,k_g,v_g per 4-head group g; attention for group g
        # runs right after its three strips, overlapping later groups' matmuls.
        with tc.tile_pool(name="ropool", bufs=3) as ropool, \
             tc.tile_pool(name="qkpool", bufs=2) as qkpool, \
             tc.tile_pool(name="apool", bufs=2) as apool, \
             tc.tile_pool(name="qkv_psum", bufs=3, space="PSUM") as psum_mm, \
             tc.tile_pool(name="at_psum", bufs=3, space="PSUM", side="right") as psum_at, \
             tc.tile_pool(name="ctx_psum", bufs=2, space="PSUM", side="right") as psum_cx:
            for g in range(4):
                qTg = qkpool.tile([128, NT, 4, 128], BF16, tag="qTg")
                kTg = qkpool.tile([128, NT, 4, 128], BF16, tag="kTg")
                v_g = qkpool.tile([128, NT, 4, 132], BF16, tag="vg")
                nc.vector.memset(v_g[:, :, :, 128:129], 1.0)
                for kind in range(3):
                    src = g + 4 * kind
                    if g == 0 and kind < 2:
                        wst = wst_pre[kind]
                    else:
                        wst = qkwpool.tile([128, NK, 512], BF16, tag="w_qkv")
                        nc.gpsimd.dma_start(wst[:], wqkv_in[src, :, :, :])
                    for t in range(NT):
                        ps = psum_mm.tile([128, 512], F32, tag="ps_qkv")
                        for kk in range(NK):
                            nc.tensor.matmul(ps[:], xqT[:, t, kk, :], wst[:, kk, :],
                                             start=(kk == 0), stop=(kk == NK - 1))
                        if kind == 2:
                            nc.scalar.copy(v_g[:, t, :, 0:128],
                                           ps[:].rearrange("p (c f) -> p c f", c=4))
                        else:
                            qsc = ropool.tile([128, 4, 128], BF16, tag="qsc")
                            nc.scalar.copy(qsc[:], ps[:].rearrange("p (c f) -> p c f", c=4))
                            p1, p2 = qsc[:, :, 0:64], qsc[:, :, 64:128]
                            cosd = cs_b[:, 0, t, :].rearrange("p (c f) -> p c f", c=4)
                            sind = cs_b[:, 1, t, :].rearrange("p (c f) -> p c f", c=4)
                            t1 = ropool.tile([128, 4, 64], F32, tag="rt1")
                            t2 = ropool.tile([128, 4, 64], F32, tag="rt2")
                            rot = ropool.tile([128, 4, 128], BF16, tag="rot")
                            nc.vector.tensor_tensor(t1[:], p1, cosd, OP.mult)
                            nc.gpsimd.tensor_tensor(t2[:], p2, sind, OP.mult)
                            nc.vector.tensor_tensor(rot[:, :, 0:64], t1[:], t2[:], OP.subtract)
                            nc.vector.tensor_tensor(t1[:], p2, cosd, OP.mult)
                            nc.gpsimd.tensor_tensor(t2[:], p1, sind, OP.mult)
                            nc.vector.tensor_tensor(rot[:, :, 64:128], t1[:], t2[:], OP.add)
                            dst = qTg if kind == 0 else kTg
                            nc.sync.dma_start_transpose(
                                dst[:, t, :, :], rot[:].rearrange("p c f -> p (c f)"))
                for blk in range(2):
                    for hl in range(4):
                        expT = [None] * 4
                        for kt in range(4):
                            qn = 512 - kt * 128
                            pss = psum_at.tile([128, 512], F32, tag="ps_sc")
                            nc.tensor.matmul(
                                pss[:, 0:qn],
                                kTg[:, blk * 4 + kt, hl, :],
                                qTg[:, blk * 4 + kt:(blk + 1) * 4, hl, :],
                                start=True, stop=True)
                            ex = apool.tile([128, 512], BF16, tag=f"expT{kt}")
                            nc.scalar.activation(ex[:, 0:qn], pss[:, 0:qn], ACTF.Exp,
                                                 scale=c_att)
                            nc.gpsimd.affine_select(
                                out=ex[:, 0:128], in_=ex[:, 0:128],
                                compare_op=OP.is_ge, fill=0.0,
                                base=0, pattern=[[1, 128]], channel_multiplier=-1)
                            expT[kt] = ex
                        for qt in range(4):
                            psc = psum_cx.tile([128, 132], F32, tag="ps_ctx")
                            for kt in range(qt + 1):
                                nc.tensor.matmul(psc[:, 0:129],
                                                 expT[kt][:, (qt - kt) * 128:(qt - kt) * 128 + 128],
                                                 v_g[:, blk * 4 + kt, hl, 0:129],
                                                 start=(kt == 0), stop=(kt == qt))
                            rl = apool.tile([128, 1], F32, tag="rl")
                            nc.vector.reciprocal(rl[:], psc[:, 128:129])
                            nc.vector.tensor_scalar_mul(
                                ctx_sb[:, blk * 4 + qt, 4 * g + hl, :],
                                psc[:, 0:128], rl[:])
        qkwpool.release()
        slot1.release()

        # ---------- P4: ctx act-qdq + transpose ----------
        ctxq_pool = tc.alloc_tile_pool(name="ctxq_pool", bufs=1, side="right")
        ctxqT = ctxq_pool.tile([128, NT, NK, 128], BF16)
        with tc.tile_pool(name="cqpool", bufs=2) as cqpool:
            for t in range(NT):
                src = ctx_sb[:, t, :, :].rearrange("p c f -> p (c f)")
                amax = cqpool.tile([128, 1], F32, tag="amax_cq")
                nc.vector.tensor_reduce(amax[:], src, AX.X, OP.max,
                                        apply_absolute_value=True)
                cq = cqpool.tile([128, H], BF16, tag="cq")
                _quant_pair(nc, cqpool, src, H, amax[:], sc[:, t:t + 1],
                            dqc[:, t:t + 1], cq[:], magic_ap, "cq")
                nc.sync.dma_start_transpose(ctxqT[:, t, :, :], cq[:])
        ctx_pool.release()

        # ---------- P5: o_proj + residual -> x1_d (+ ffn-norm ssq fused) ----------
        # ---------- P6: ffn rmsnorm (slim) + transpose ----------
        hnT_pool = tc.alloc_tile_pool(name="hnT_pool", bufs=1)
        hnT = hnT_pool.tile([128, NT, NK, 128], BF16)
        with tc.tile_pool(name="opool", bufs=3) as opool, \
             tc.tile_pool(name="owpool", bufs=2) as owpool, \
             tc.tile_pool(name="n2pool", bufs=2) as n2pool, \
             tc.tile_pool(name="n2wpool", bufs=1) as n2wpool, \
             tc.tile_pool(name="o_psum", bufs=3, space="PSUM") as psum_o:
            fnw_b = n2wpool.tile([128, H], F32, tag="normw2")
            ap2 = fnw_in[:]
            nc.gpsimd.dma_start(out=fnw_b[:], in_=bass.AP(
                tensor=ap2.tensor, offset=ap2.offset, ap=[[0, 128]] + list(ap2.ap)))
            for nn in range(4):
                wst = owpool.tile([128, NK, 512], BF16, tag="wo_st")
                nc.gpsimd.dma_start(wst[:], wo_in[nn, :, :, :])
                for t in range(NT):
                    ps = psum_o.tile([128, 512], F32, tag="ps_o")
                    for kk in range(NK):
                        nc.tensor.matmul(ps[:], ctxqT[:, t, kk, :], wst[:, kk, :],
                                         start=(kk == 0), stop=(kk == NK - 1))
                    xs = opool.tile([128, 512], F32, tag="xs")
                    nc.scalar.dma_start(xs[:], x_in[t * 128:(t + 1) * 128,
                                                    nn * 512:(nn + 1) * 512])
                    tmp = opool.tile([128, 512], F32, tag="o_tmp")
                    nc.scalar.activation(tmp[:], ps[:], ACTF.Identity, scale=c_o)
                    x1s = opool.tile([128, 512], F32, tag="x1s")
                    nc.vector.tensor_tensor(x1s[:], tmp[:], xs[:], OP.add)
                    sqt = opool.tile([128, 512], F32, tag="o_sq")
                    nc.vector.tensor_tensor(sqt[:], x1s[:], x1s[:], OP.mult)
                    nc.vector.tensor_reduce(ssq_parts[:, t, nn:nn + 1], sqt[:],
                                            AX.X, OP.add)
                    nc.scalar.dma_start(x1_d[t, :, nn * 512:(nn + 1) * 512], x1s[:])
            for t in range(NT):
                x1_t = n2pool.tile([128, H], F32, tag="x1n")
                nc.scalar.dma_start(x1_t[:], x1_d[t, :, :])
                ssq = n2pool.tile([128, 1], F32, tag="ssq2")
                nc.vector.tensor_reduce(ssq[:], ssq_parts[:, t, :], AX.X, OP.add)
                msq = n2pool.tile([128, 1], F32, tag="msq2")
                nc.vector.tensor_scalar(msq[:], ssq[:], 1.0 / H, EPS, OP.mult, OP.add)
                sd = n2pool.tile([128, 1], F32, tag="sd2")
                nc.scalar.activation(sd[:], msq[:], ACTF.Sqrt)
                rstd = n2pool.tile([128, 1], F32, tag="rstd2")
                nc.vector.reciprocal(rstd[:], sd[:])
                hn_t = n2pool.tile([128, H], BF16, tag="hn_t")
                nc.vector.tensor_scalar_mul(hn_t[:], x1_t[:], rstd[:])
                nc.vector.tensor_tensor(hn_t[:], hn_t[:], fnw_b[:], OP.mult)
                nc.sync.dma_start_transpose(hnT[:, t, :, :], hn_t[:])
        ctxq_pool.release()

        # ---------- P7: ffn up + silu*val -> act_d ----------
        with tc.tile_pool(name="upool", bufs=2) as upool, \
             tc.tile_pool(name="fpool", bufs=3) as fpool, \
             tc.tile_pool(name="up_psum", bufs=2, space="PSUM", side="right") as psum_up, \
             tc.tile_pool(name="upv_psum", bufs=2, space="PSUM", side="right") as psum_upv:
            for i in range(8):   # paired gate/val strips of 512
                wgr = upool.tile([128, NK, 512], BF16, tag="w_up")
                nc.gpsimd.dma_start(wgr[:], wup_in[i, :, :, :])
                wvr = upool.tile([128, NK, 512], BF16, tag="w_up")
                nc.gpsimd.dma_start(wvr[:], wup_in[8 + i, :, :, :])
                for t in range(NT):
                    psg = psum_up.tile([128, 512], F32, tag="ps_g")
                    for kk in range(NK):
                        nc.tensor.matmul(psg[:], hnT[:, t, kk, :], wgr[:, kk, :],
                                         start=(kk == 0), stop=(kk == NK - 1))
                    psv = psum_upv.tile([128, 512], F32, tag="ps_v")
                    for kk in range(NK):
                        nc.tensor.matmul(psv[:], hnT[:, t, kk, :], wvr[:, kk, :],
                                         start=(kk == 0), stop=(kk == NK - 1))
                    sgm = fpool.tile([128, 512], F32, tag="sgm")
                    nc.scalar.activation(sgm[:], psg[:], ACTF.Sigmoid)
                    sg = fpool.tile([128, 512], F32, tag="sg")
                    nc.vector.tensor_tensor(sg[:], sgm[:], psg[:], OP.mult)
                    av = fpool.tile([128, 512], BF16, tag="av")
                    nc.vector.tensor_tensor(av[:], sg[:], psv[:], OP.mult)
                    nc.sync.dma_start(act_d[t, :, i * 512:(i + 1) * 512], av[:])
                    rmax = fpool.tile([128, 1], F32, tag="rmax")
                    nc.vector.tensor_reduce(rmax[:], av[:], AX.X, OP.max,
                                            apply_absolute_value=True)
                    nc.vector.tensor_tensor(amax_av[:, t:t + 1], amax_av[:, t:t + 1],
                                            rmax[:], OP.max)

        hnT_pool.release()

        # ---------- P8: ffn act-qdq + transpose -> actqT (SBUF) ----------
        # ---------- P9: ffn down + residual -> out ----------
        actq_pool = tc.alloc_tile_pool(name="actq_pool", bufs=1)
        actqT = actq_pool.tile([128, NT, NKI, 128], BF16)
        with tc.tile_pool(name="aqpool", bufs=2) as aqpool, \
             tc.tile_pool(name="dpool", bufs=2) as dpool, \
             tc.tile_pool(name="dopool", bufs=3) as dopool, \
             tc.tile_pool(name="dn_psum", bufs=4, space="PSUM") as psum_dn:
            # prefetch first down-weight strip ahead of the quant pass
            wdn_tiles = {}
            for half in range(2):
                w = dpool.tile([128, 16, 512], BF16, tag=f"w_dn_{half}")
                nc.gpsimd.dma_start(w[:], wdn_in[0, half, :, :, :])
                wdn_tiles[(0, half)] = w
            for t in range(NT):
                amc = aqpool.tile([128, 1], F32, tag="amc_a")
                nc.vector.tensor_scalar_max(amc[:], amax_av[:, t:t + 1], 1e-5)
                rec = aqpool.tile([128, 1], F32, tag="rec_a")
                nc.vector.reciprocal(rec[:], amc[:])
                nc.vector.tensor_scalar_mul(sa[:, t:t + 1], rec[:], 127.0)
                nc.vector.tensor_scalar_mul(dqa[:, t:t + 1], amc[:], 1.0 / 127.0)
                for hf in range(4):
                    at = aqpool.tile([128, 1024], BF16, tag="at")
                    nc.gpsimd.dma_start(at[:], act_d[t, :, hf * 1024:(hf + 1) * 1024])
                    mg = aqpool.tile([128, 1024], F32, tag="mg_a")
                    nc.scalar.activation(mg[:], at[:], ACTF.Identity, bias=magic_ap,
                                         scale=sa[:, t:t + 1])
                    aq = aqpool.tile([128, 1024], BF16, tag="aq")
                    nc.vector.tensor_scalar(aq[:], mg[:], float(MAGIC), dqa[:, t:t + 1],
                                            OP.subtract, OP.mult)
                    dmae = nc.sync if hf % 2 == 0 else nc.scalar
                    dmae.dma_start_transpose(actqT[:, t, hf * 8:(hf + 1) * 8, :], aq[:])
            for nn in range(4):
                if nn > 0:
                    for half in range(2):
                        w = dpool.tile([128, 16, 512], BF16, tag=f"w_dn_{half}")
                        nc.gpsimd.dma_start(w[:], wdn_in[nn, half, :, :, :])
                        wdn_tiles[(nn, half)] = w
                wsa = wdn_tiles[(nn, 0)]
                wsb = wdn_tiles[(nn, 1)]
                for t in range(NT):
                    ps = psum_dn.tile([128, 512], F32, tag="ps_dn")
                    for kk in range(NKI):
                        w = wsa[:, kk, :] if kk < 16 else wsb[:, kk - 16, :]
                        nc.tensor.matmul(ps[:], actqT[:, t, kk, :], w,
                                         start=(kk == 0), stop=(kk == NKI - 1))
                    x1_t = dopool.tile([128, 512], F32, tag="x1_re")
                    nc.scalar.dma_start(x1_t[:], x1_d[t, :, nn * 512:(nn + 1) * 512])
                    tmp = dopool.tile([128, 512], F32, tag="d_tmp")
                    nc.scalar.activation(tmp[:], ps[:], ACTF.Identity, scale=c_dn)
                    ot = dopool.tile([128, 512], F32, tag="ot")
                    nc.vector.tensor_tensor(ot[:], tmp[:], x1_t[:], OP.add)
                    nc.scalar.dma_start(out_d[t * 128:(t + 1) * 128,
                                              nn * 512:(nn + 1) * 512], ot[:])
        actq_pool.release()
        cs_pool.release()
        perm.release()

    nc.compile()
    return nc


_NC_CACHE = None


def _get_nc():
    global _NC_CACHE
    if _NC_CACHE is None:
        _NC_CACHE = build_program()
    return _NC_CACHE


def _ternarize(w):
    """Reference _weight_quant: returns (ternary float {-1,0,1}, dqw scale)."""
    w = np.asarray(w, np.float32)
    m = np.maximum(np.mean(np.abs(w), dtype=np.float32), np.float32(1e-5))
    scale = np.float32(1.0) / m
    tern = np.clip(np.round(w * scale), -1.0, 1.0).astype(np.float32)
    return tern, float(m)


def _strip_layout(w_t, n_strips, nk):
    """[in_feats, out_feats] -> [n_strips, 128, nk, 512] (strip s covers out
    cols s*512..). w_t is the transposed weight [in, out]."""
    infeat, outfeat = w_t.shape
    assert infeat == nk * 128 and outfeat == n_strips * 512
    # [nk, 128, n_strips, 512] -> [n_strips, 128, nk, 512]
    v = w_t.reshape(nk, 128, n_strips, 512)
    return np.ascontiguousarray(v.transpose(2, 1, 0, 3))


def _host_inputs(x, attn_norm_w, ffn_norm_w, qkv_w, o_w, ffn_up_w, ffn_down_w):
    x = np.ascontiguousarray(np.asarray(x, np.float32))
    anw = np.ascontiguousarray(np.asarray(attn_norm_w, np.float32))
    fnw = np.ascontiguousarray(np.asarray(ffn_norm_w, np.float32))

    tern_qkv, dqw_qkv = _ternarize(qkv_w)
    tern_o, dqw_o = _ternarize(o_w)
    tern_dn, dqw_dn = _ternarize(ffn_down_w)

    wqkv_sh = _strip_layout(tern_qkv.T, 12, NK).astype(ml_dtypes.bfloat16)
    wo_sh = _strip_layout(tern_o.T, 4, NK).astype(ml_dtypes.bfloat16)
    wup_sh = _strip_layout(np.asarray(ffn_up_w, np.float32).T, 16, NK) \
        .astype(ml_dtypes.bfloat16)
    wdn_sh = _strip_layout(tern_dn.T, 4, NKI).astype(ml_dtypes.bfloat16) \
        .reshape(4, 128, 2, 16, 512).transpose(0, 2, 1, 3, 4)
    wdn_sh = np.ascontiguousarray(wdn_sh)

    consts = np.zeros(8, np.float32)
    consts[0] = dqw_qkv * dqw_qkv * (HD ** -0.5)
    consts[1] = dqw_qkv * dqw_o
    consts[2] = dqw_dn

    # rope tables: cs[p, 0/1, t, 4*64] bf16, replicated x4 for the 4-head strips
    inv = 1.0 / (THETA ** (np.arange(0, HD, 2, dtype=np.float32) / HD))
    tpos = np.arange(S, dtype=np.float32)
    fr = np.outer(tpos, inv)                     # [S, 64]
    cos = np.tile(np.cos(fr), (1, 4))            # [S, 256]
    sin = np.tile(np.sin(fr), (1, 4))

    in_maps = []
    for c in range(NCORES):
        b = c // 4
        t0 = (c % 4) * R
        # [R, 256] -> [NT, 128, 256] -> [128, NT, 256]
        cs = np.stack([cos[t0:t0 + R].reshape(NT, 128, 256).transpose(1, 0, 2),
                       sin[t0:t0 + R].reshape(NT, 128, 256).transpose(1, 0, 2)],
                      axis=1)                    # [128, 2, NT, 256]
        in_maps.append({
            "x_sh": np.ascontiguousarray(x[b, t0:t0 + R, :]),
            "cs_sh": np.ascontiguousarray(cs).astype(ml_dtypes.bfloat16),
            "attn_norm_w": anw, "ffn_norm_w": fnw, "consts": consts,
            "wqkv_sh": wqkv_sh, "wo_sh": wo_sh, "wup_sh": wup_sh, "wdn_sh": wdn_sh,
        })
    return in_maps


def run(trace=False, **inputs):
    nc = _get_nc()
    in_maps = _host_inputs(**inputs)
    res = run_bass_kernel_spmd(nc, in_maps, list(range(NCORES)), trace=trace)
    out = np.empty((B, S, H), np.float32)
    for c in range(NCORES):
        b = c // 4
        t0 = (c % 4) * R
        out[b, t0:t0 + R, :] = res.results[c]["out_sh"]
    return out, res


def kernel(**inputs):
    out, _ = run(trace=False, **inputs)
    return out


# revision 44
# speedup vs baseline: 1.2036x; 1.0212x over previous
"""Trainium2 Bass kernel for nn_BlockAttentionResidual (block attention + BitNet-style quantized MLP).

Sharding: sequence-block data parallelism, zero collectives. Block attention is
independent per 512-token block, so each of the 8 cores owns 1024 contiguous
tokens (2 blocks) of one batch element and runs the whole layer on them.
  core c -> batch c//4, tokens [(c%4)*1024, (c%4+1)*1024)

Weights are static parameters: ternarization (per-tensor mean|w| scale, exact
reference semantics) is host-side preprocessing; the ternary {-1,0,1} values are
exact in bf16.  Per-tensor dequant scalars fold into three constants shipped as
a tiny input tensor (exp-scale for attention, psum-eviction scales for o_proj /
ffn_down), so the device does no dequant bookkeeping: activations are
quantize-dequantized in one fused ACT+DVE pair per tile and all matmuls run on
bf16 operands with fp32 PSUM accumulation.

All DRAM layouts are pre-tiled host-side so every DMA line is >=1KB contiguous
per partition (the previous kernel was DMA-descriptor-bound: 600k descriptors
averaging 800B).
"""

import numpy as np
import ml_dtypes

import concourse.bass as bass
import concourse.mybir as mybir
import concourse.tile as tile
from concourse import bacc
from concourse.bass_utils import run_bass_kernel_spmd

F32 = mybir.dt.float32
BF16 = mybir.dt.bfloat16
AX = mybir.AxisListType
OP = mybir.AluOpType
ACTF = mybir.ActivationFunctionType

# model dims
H = 2048
NH = 16
HD = 128
NB = 8
INTER = 4096        # 2*H
EPS = 1e-5
THETA = 10000.0
B, S = 2, 4096
BT = 512            # tokens per attention block
NCORES = 8
R = 1024            # tokens per core
NT = R // 128       # 8 token tiles per core
NK = H // 128       # 16 k-tiles of the hidden dim
NKI = INTER // 128  # 32 k-tiles of the intermediate dim
MAGIC = np.float32(1.5 * 2 ** 23)   # fp32 round-to-nearest-even magic


def _quant_pair(nc, pool, src_ap, ncols, amax_ap, s_store, dq_store, out_bf,
                magic_ap, tag):
    """Quantize-dequantize src_ap [128, ncols] onto the int8 grid:
    out_bf = round(src*127/amax) * amax/127 in bf16.  amax_ap: [128,1] f32."""
    amc = pool.tile([128, 1], F32, tag=f"amc_{tag}")
    nc.vector.tensor_scalar_max(amc[:], amax_ap, 1e-5)
    rec = pool.tile([128, 1], F32, tag=f"rec_{tag}")
    nc.vector.reciprocal(rec[:], amc[:])
    nc.vector.tensor_scalar_mul(s_store, rec[:], 127.0)
    nc.vector.tensor_scalar_mul(dq_store, amc[:], 1.0 / 127.0)
    mg = pool.tile([128, ncols], F32, tag=f"mg_{tag}")
    nc.scalar.activation(mg[:], src_ap, ACTF.Identity, bias=magic_ap, scale=s_store)
    nc.vector.tensor_scalar(out_bf, mg[:], float(MAGIC), dq_store, OP.subtract,
                            OP.mult)


def build_program():
    nc = bacc.Bacc(None, target_bir_lowering=False)

    # ---- I/O (all layouts pre-tiled on host) ----
    x_in = nc.declare_dram_parameter("x_sh", [R, H], F32, isOutput=False)
    cs_in = nc.declare_dram_parameter("cs_sh", [128, 2, NT, 256], BF16, isOutput=False)
    anw_in = nc.declare_dram_parameter("attn_norm_w", [H], F32, isOutput=False)
    fnw_in = nc.declare_dram_parameter("ffn_norm_w", [H], F32, isOutput=False)
    consts_in = nc.declare_dram_parameter("consts", [8], F32, isOutput=False)
    # weight strips: [strip, 128 kpart, n_ktiles, 512 outcols] bf16
    wqkv_in = nc.declare_dram_parameter("wqkv_sh", [12, 128, NK, 512], BF16, isOutput=False)
    wo_in = nc.declare_dram_parameter("wo_sh", [4, 128, NK, 512], BF16, isOutput=False)
    wup_in = nc.declare_dram_parameter("wup_sh", [16, 128, NK, 512], BF16, isOutput=False)
    # ffn_down strips split in two k-halves: [strip, half, 128, 16, 512]
    wdn_in = nc.declare_dram_parameter("wdn_sh", [4, 2, 128, 16, 512], BF16, isOutput=False)
    out_d = nc.declare_dram_parameter("out_sh", [R, H], F32, isOutput=True)

    # ---- internal DRAM scratch ----
    x1_d = nc.dram_tensor("x1_d", [NT, 128, H], F32)
    act_d = nc.dram_tensor("act_d", [NT, 128, INTER], BF16)      # silu(g)*v rows

    with tile.TileContext(nc) as tc:
        perm = tc.alloc_tile_pool(name="perm", bufs=1)
        magic_t = perm.tile([128, 1], F32)
        nc.vector.memset(magic_t[:], float(MAGIC))
        magic_ap = magic_t[:]
        consts_b = perm.tile([128, 8], F32)
        ap0 = consts_in[:]
        nc.gpsimd.dma_start(out=consts_b[:], in_=bass.AP(
            tensor=ap0.tensor, offset=ap0.offset, ap=[[0, 128]] + list(ap0.ap)))
        c_att = consts_b[:, 0:1]   # dqw_qkv^2 * HD^-0.5
        c_o = consts_b[:, 1:2]     # dqw_qkv * dqw_o
        c_dn = consts_b[:, 2:3]    # dqw_dn
        # per-token quant scales (s = 127/amax, dq = amax/127)
        s1 = perm.tile([128, NT], F32)
        dq1 = perm.tile([128, NT], F32)
        sc = perm.tile([128, NT], F32)
        dqc = perm.tile([128, NT], F32)
        sa = perm.tile([128, NT], F32)
        dqa = perm.tile([128, NT], F32)
        amax_av = perm.tile([128, NT], F32)
        nc.vector.memset(amax_av[:], 0.0)

        ssq_parts = perm.tile([128, NT, 4], F32)

        cs_pool = tc.alloc_tile_pool(name="cs_pool", bufs=1)
        cs_b = cs_pool.tile([128, 2, NT, 256], BF16)
        nc.gpsimd.dma_start(cs_b[:], cs_in[:])

        ctx_pool = tc.alloc_tile_pool(name="ctx_pool", bufs=1)
        ctx_sb = ctx_pool.tile([128, NT, NH, 128], BF16)
        slot1 = tc.alloc_tile_pool(name="slot1", bufs=1)
        xqT = slot1.tile([128, NT, NK, 128], BF16)

        # qkv weight pool opened early so the first strips prefetch during P1
        qkwpool = tc.alloc_tile_pool(name="qkwpool", bufs=2)
        wst_pre = []
        for kind in range(2):
            wst = qkwpool.tile([128, NK, 512], BF16, tag="w_qkv")
            nc.gpsimd.dma_start(wst[:], wqkv_in[4 * kind, :, :, :])
            wst_pre.append(wst)

        # ---------- P1: attn rmsnorm + act-qdq + transpose ----------
        with tc.tile_pool(name="npool", bufs=2) as npool, \
             tc.tile_pool(name="nwpool", bufs=1) as nwpool:
            anw_b = nwpool.tile([128, H], F32, tag="normw")
            ap1 = anw_in[:]
            nc.gpsimd.dma_start(out=anw_b[:], in_=bass.AP(
                tensor=ap1.tensor, offset=ap1.offset, ap=[[0, 128]] + list(ap1.ap)))
            for t in range(NT):
                xt = npool.tile([128, H], F32, tag="xt")
                nc.scalar.dma_start(xt[:], x_in[t * 128:(t + 1) * 128, :])
                ssq = npool.tile([128, 1], F32, tag="ssq")
                junk = npool.tile([128, H], BF16, tag="njunk")
                nc.scalar.activation(junk[:], xt[:], ACTF.Square, accum_out=ssq[:])
                msq = npool.tile([128, 1], F32, tag="msq")
                nc.vector.tensor_scalar(msq[:], ssq[:], 1.0 / H, EPS, OP.mult, OP.add)
                sd = npool.tile([128, 1], F32, tag="sd")
                nc.scalar.activation(sd[:], msq[:], ACTF.Sqrt)
                rstd = npool.tile([128, 1], F32, tag="rstd")
                nc.vector.reciprocal(rstd[:], sd[:])
                h_t = npool.tile([128, H], F32, tag="h_t")
                nc.vector.tensor_scalar_mul(h_t[:], xt[:], rstd[:])
                nc.gpsimd.tensor_tensor(h_t[:], h_t[:], anw_b[:], OP.mult)
                amax = npool.tile([128, 1], F32, tag="amax_n1")
                nc.vector.tensor_reduce(amax[:], h_t[:], AX.X, OP.max,
                                        apply_absolute_value=True)
                xq = npool.tile([128, H], BF16, tag="xq")
                _quant_pair(nc, npool, h_t[:], H, amax[:], s1[:, t:t + 1],
                            dq1[:, t:t + 1], xq[:], magic_ap, "n1")
                nc.sync.dma_start_transpose(xqT[:, t, :, :], xq[:])

        # ---------- P2+P3: qkv matmul + rope interleaved with block attention ----------
        # strips ordered q_g,k_g,v_g per 4-head group g; attention for group g
        # runs right after its three strips, overlapping later groups' matmuls.
        with tc.tile_pool(name="ropool", bufs=3) as ropool, \
             tc.tile_pool(name="qkpool", bufs=2) as qkpool, \
             tc.tile_pool(name="apool", bufs=2) as apool, \
             tc.tile_pool(name="qkv_psum", bufs=3, space="PSUM") as psum_mm, \
             tc.tile_pool(name="at_psum", bufs=3, space="PSUM", side="right") as psum_at, \
             tc.tile_pool(name="ctx_psum", bufs=2, space="PSUM", side="right") as psum_cx:
            for g in range(4):
                qTg = qkpool.tile([128, NT, 4, 128], BF16, tag="qTg")
                kTg = qkpool.tile([128, NT, 4, 128], BF16, tag="kTg")
                v_g = qkpool.tile([128, NT, 4, 132], BF16, tag="vg")
                nc.vector.memset(v_g[:, :, :, 128:129], 1.0)
                for kind in range(3):
                    src = g + 4 * kind
                    if g == 0 and kind < 2:
                        wst = wst_pre[kind]
                    else:
                        wst = qkwpool.tile([128, NK, 512], BF16, tag="w_qkv")
                        nc.gpsimd.dma_start(wst[:], wqkv_in[src, :, :, :])
                    for t in range(NT):
                        ps = psum_mm.tile([128, 512], F32, tag="ps_qkv")
                        for kk in range(NK):
                            nc.tensor.matmul(ps[:], xqT[:, t, kk, :], wst[:, kk, :],
                                             start=(kk == 0), stop=(kk == NK - 1))
                        if kind == 2:
                            nc.scalar.copy(v_g[:, t, :, 0:128],
                                           ps[:].rearrange("p (c f) -> p c f", c=4))
                        else:
                            qsc = ropool.tile([128, 4, 128], BF16, tag="qsc")
                            nc.scalar.copy(qsc[:], ps[:].rearrange("p (c f) -> p c f", c=4))
                            p1, p2 = qsc[:, :, 0:64], qsc[:, :, 64:128]
                            cosd = cs_b[:, 0, t, :].rearrange("p (c f) -> p c f", c=4)
                            sind = cs_b[:, 1, t, :].rearrange("p (c f) -> p c f", c=4)
                            t1 = ropool.tile([128, 4, 64], F32, tag="rt1")
                            t2 = ropool.tile([128, 4, 64], F32, tag="rt2")
                            rot = ropool.tile([128, 4, 128], BF16, tag="rot")
                            nc.vector.tensor_tensor(t1[:], p1, cosd, OP.mult)
                            nc.gpsimd.tensor_tensor(t2[:], p2, sind, OP.mult)
                            nc.vector.tensor_tensor(rot[:, :, 0:64], t1[:], t2[:], OP.subtract)
                            nc.vector.tensor_tensor(t1[:], p2, cosd, OP.mult)
                            nc.gpsimd.tensor_tensor(t2[:], p1, sind, OP.mult)
                            nc.vector.tensor_tensor(rot[:, :, 64:128], t1[:], t2[:], OP.add)
                            dst = qTg if kind == 0 else kTg
                            nc.sync.dma_start_transpose(
                                dst[:, t, :, :], rot[:].rearrange("p c f -> p (c f)"))
                for blk in range(2):
                    for hl in range(4):
                        expT = [None] * 4
                        for kt in range(4):
                            qn = 512 - kt * 128
                            pss = psum_at.tile([128, 512], F32, tag="ps_sc")
                            nc.tensor.matmul(
                                pss[:, 0:qn],
                                kTg[:, blk * 4 + kt, hl, :],
                                qTg[:, blk * 4 + kt:(blk + 1) * 4, hl, :],
                                start=True, stop=True)
                            ex = apool.tile([128, 512], BF16, tag=f"expT{kt}")
                            nc.scalar.activation(ex[:, 0:qn], pss[:, 0:qn], ACTF.Exp,
                                                 scale=c_att)
                            nc.gpsimd.affine_select(
                                out=ex[:, 0:128], in_=ex[:, 0:128],
                                compare_op=OP.is_ge, fill=0.0,
                                base=0, pattern=[[1, 128]], channel_multiplier=-1)
                            expT[kt] = ex
                        for qt in range(4):
                            psc = psum_cx.tile([128, 132], F32, tag="ps_ctx")
                            for kt in range(qt + 1):
                                nc.tensor.matmul(psc[:, 0:129],
                                                 expT[kt][:, (qt - kt) * 128:(qt - kt) * 128 + 128],
                                                 v_g[:, blk * 4 + kt, hl, 0:129],
                                                 start=(kt == 0), stop=(kt == qt))
                            rl = apool.tile([128, 1], F32, tag="rl")
                            nc.vector.reciprocal(rl[:], psc[:, 128:129])
                            nc.vector.tensor_scalar_mul(
                                ctx_sb[:, blk * 4 + qt, 4 * g + hl, :],
                                psc[:, 0:128], rl[:])
        qkwpool.release()
        slot1.release()

        # ---------- P4: ctx act-qdq + transpose ----------
        ctxq_pool = tc.alloc_tile_pool(name="ctxq_pool", bufs=1, side="right")
        ctxqT = ctxq_pool.tile([128, NT, NK, 128], BF16)
        with tc.tile_pool(name="cqpool", bufs=2) as cqpool:
            for t in range(NT):
                src = ctx_sb[:, t, :, :].rearrange("p c f -> p (c f)")
                amax = cqpool.tile([128, 1], F32, tag="amax_cq")
                nc.vector.tensor_reduce(amax[:], src, AX.X, OP.max,
                                        apply_absolute_value=True)
                cq = cqpool.tile([128, H], BF16, tag="cq")
                _quant_pair(nc, cqpool, src, H, amax[:], sc[:, t:t + 1],
                            dqc[:, t:t + 1], cq[:], magic_ap, "cq")
                nc.sync.dma_start_transpose(ctxqT[:, t, :, :], cq[:])
        ctx_pool.release()

        # ---------- P5: o_proj + residual -> x1_d (+ ffn-norm ssq fused) ----------
        # ---------- P6: ffn rmsnorm (slim) + transpose ----------
        hnT_pool = tc.alloc_tile_pool(name="hnT_pool", bufs=1)
        hnT = hnT_pool.tile([128, NT, NK, 128], BF16)
        with tc.tile_pool(name="opool", bufs=3) as opool, \
             tc.tile_pool(name="owpool", bufs=2) as owpool, \
             tc.tile_pool(name="n2pool", bufs=2) as n2pool, \
             tc.tile_pool(name="n2wpool", bufs=1) as n2wpool, \
             tc.tile_pool(name="o_psum", bufs=3, space="PSUM") as psum_o:
            fnw_b = n2wpool.tile([128, H], F32, tag="normw2")
            ap2 = fnw_in[:]
            nc.gpsimd.dma_start(out=fnw_b[:], in_=bass.AP(
                tensor=ap2.tensor, offset=ap2.offset, ap=[[0, 128]] + list(ap2.ap)))
            for nn in range(4):
                wst = owpool.tile([128, NK, 512], BF16, tag="wo_st")
                nc.gpsimd.dma_start(wst[:], wo_in[nn, :, :, :])
                for t in range(NT):
                    ps = psum_o.tile([128, 512], F32, tag="ps_o")
                    for kk in range(NK):
                        nc.tensor.matmul(ps[:], ctxqT[:, t, kk, :], wst[:, kk, :],
                                         start=(kk == 0), stop=(kk == NK - 1))
                    xs = opool.tile([128, 512], F32, tag="xs")
                    nc.scalar.dma_start(xs[:], x_in[t * 128:(t + 1) * 128,
                                                    nn * 512:(nn + 1) * 512])
                    tmp = opool.tile([128, 512], F32, tag="o_tmp")
                    nc.scalar.activation(tmp[:], ps[:], ACTF.Identity, scale=c_o)
                    x1s = opool.tile([128, 512], F32, tag="x1s")
                    nc.vector.tensor_tensor(x1s[:], tmp[:], xs[:], OP.add)
                    sqt = opool.tile([128, 512], F32, tag="o_sq")
                    nc.vector.tensor_tensor(sqt[:], x1s[:], x1s[:], OP.mult)
                    nc.vector.tensor_reduce(ssq_parts[:, t, nn:nn + 1], sqt[:],
                                            AX.X, OP.add)
                    nc.scalar.dma_start(x1_d[t, :, nn * 512:(nn + 1) * 512], x1s[:])
            for t in range(NT):
                x1_t = n2pool.tile([128, H], F32, tag="x1n")
                nc.sync.dma_start(x1_t[:], x1_d[t, :, :])
                ssq = n2pool.tile([128, 1], F32, tag="ssq2")
                nc.vector.tensor_reduce(ssq[:], ssq_parts[:, t, :], AX.X, OP.add)
                msq = n2pool.tile([128, 1], F32, tag="msq2")
                nc.vector.tensor_scalar(msq[:], ssq[:], 1.0 / H, EPS, OP.mult, OP.add)
                sd = n2pool.tile([128, 1], F32, tag="sd2")
                nc.scalar.activation(sd[:], msq[:], ACTF.Sqrt)
                rstd = n2pool.tile([128, 1], F32, tag="rstd2")
                nc.vector.reciprocal(rstd[:], sd[:])
                hn_t = n2pool.tile([128, H], BF16, tag="hn_t")
                nc.vector.tensor_scalar_mul(hn_t[:], x1_t[:], rstd[:])
                nc.vector.tensor_tensor(hn_t[:], hn_t[:], fnw_b[:], OP.mult)
                nc.sync.dma_start_transpose(hnT[:, t, :, :], hn_t[:])
        ctxq_pool.release()

        # ---------- P7: ffn up + silu*val -> act_d ----------
        with tc.tile_pool(name="upool", bufs=2) as upool, \
             tc.tile_pool(name="fpool", bufs=3) as fpool, \
             tc.tile_pool(name="up_psum", bufs=3, space="PSUM", side="right") as psum_up, \
             tc.tile_pool(name="upv_psum", bufs=3, space="PSUM", side="right") as psum_upv:
            for i in range(8):   # paired gate/val strips of 512
                wgr = upool.tile([128, NK, 512], BF16, tag="w_up")
                nc.gpsimd.dma_start(wgr[:], wup_in[i, :, :, :])
                wvr = upool.tile([128, NK, 512], BF16, tag="w_up")
                nc.gpsimd.dma_start(wvr[:], wup_in[8 + i, :, :, :])
                for t in range(NT):
                    psg = psum_up.tile([128, 512], F32, tag="ps_g")
                    for kk in range(NK):
                        nc.tensor.matmul(psg[:], hnT[:, t, kk, :], wgr[:, kk, :],
                                         start=(kk == 0), stop=(kk == NK - 1))
                    psv = psum_upv.tile([128, 512], F32, tag="ps_v")
                    for kk in range(NK):
                        nc.tensor.matmul(psv[:], hnT[:, t, kk, :], wvr[:, kk, :],
                                         start=(kk == 0), stop=(kk == NK - 1))
                    sgm = fpool.tile([128, 512], F32, tag="sgm")
                    nc.scalar.activation(sgm[:], psg[:], ACTF.Sigmoid)
                    sg = fpool.tile([128, 512], F32, tag="sg")
                    nc.vector.tensor_tensor(sg[:], sgm[:], psg[:], OP.mult)
                    av = fpool.tile([128, 512], BF16, tag="av")
                    nc.vector.tensor_tensor(av[:], sg[:], psv[:], OP.mult)
                    nc.sync.dma_start(act_d[t, :, i * 512:(i + 1) * 512], av[:])
                    rmax = fpool.tile([128, 1], F32, tag="rmax")
                    nc.vector.tensor_reduce(rmax[:], av[:], AX.X, OP.max,
                                            apply_absolute_value=True)
                    nc.vector.tensor_tensor(amax_av[:, t:t + 1], amax_av[:, t:t + 1],
                                            rmax[:], OP.max)

        hnT_pool.release()

        # ---------- P8: ffn act-qdq + transpose -> actqT (SBUF) ----------
        # ---------- P9: ffn down + residual -> out ----------
        actq_pool = tc.alloc_tile_pool(name="actq_pool", bufs=1)
        actqT = actq_pool.tile([128, NT, NKI, 128], BF16)
        with tc.tile_pool(name="aqpool", bufs=2) as aqpool, \
             tc.tile_pool(name="dpool", bufs=2) as dpool, \
             tc.tile_pool(name="dopool", bufs=3) as dopool, \
             tc.tile_pool(name="dn_psum", bufs=4, space="PSUM") as psum_dn:
            # prefetch first down-weight strip ahead of the quant pass
            wdn_tiles = {}
            for half in range(2):
                w = dpool.tile([128, 16, 512], BF16, tag=f"w_dn_{half}")
                nc.gpsimd.dma_start(w[:], wdn_in[0, half, :, :, :])
                wdn_tiles[(0, half)] = w
            for t in range(NT):
                amc = aqpool.tile([128, 1], F32, tag="amc_a")
                nc.vector.tensor_scalar_max(amc[:], amax_av[:, t:t + 1], 1e-5)
                rec = aqpool.tile([128, 1], F32, tag="rec_a")
                nc.vector.reciprocal(rec[:], amc[:])
                nc.vector.tensor_scalar_mul(sa[:, t:t + 1], rec[:], 127.0)
                nc.vector.tensor_scalar_mul(dqa[:, t:t + 1], amc[:], 1.0 / 127.0)
                for hf in range(4):
                    at = aqpool.tile([128, 1024], BF16, tag="at")
                    nc.gpsimd.dma_start(at[:], act_d[t, :, hf * 1024:(hf + 1) * 1024])
                    mg = aqpool.tile([128, 1024], F32, tag="mg_a")
                    nc.scalar.activation(mg[:], at[:], ACTF.Identity, bias=magic_ap,
                                         scale=sa[:, t:t + 1])
                    aq = aqpool.tile([128, 1024], BF16, tag="aq")
                    nc.vector.tensor_scalar(aq[:], mg[:], float(MAGIC), dqa[:, t:t + 1],
                                            OP.subtract, OP.mult)
                    nc.sync.dma_start_transpose(actqT[:, t, hf * 8:(hf + 1) * 8, :],
                                                aq[:])
            for nn in range(4):
                if nn > 0:
                    for half in range(2):
                        w = dpool.tile([128, 16, 512], BF16, tag=f"w_dn_{half}")
                        nc.gpsimd.dma_start(w[:], wdn_in[nn, half, :, :, :])
                        wdn_tiles[(nn, half)] = w
                wsa = wdn_tiles[(nn, 0)]
                wsb = wdn_tiles[(nn, 1)]
                for t in range(NT):
                    ps = psum_dn.tile([128, 512], F32, tag="ps_dn")
                    for kk in range(NKI):
                        w = wsa[:, kk, :] if kk < 16 else wsb[:, kk - 16, :]
                        nc.tensor.matmul(ps[:], actqT[:, t, kk, :], w,
                                         start=(kk == 0), stop=(kk == NKI - 1))
                    x1_t = dopool.tile([128, 512], F32, tag="x1_re")
                    nc.scalar.dma_start(x1_t[:], x1_d[t, :, nn * 512:(nn + 1) * 512])
                    tmp = dopool.tile([128, 512], F32, tag="d_tmp")
                    nc.scalar.activation(tmp[:], ps[:], ACTF.Identity, scale=c_dn)
                    ot = dopool.tile([128, 512], F32, tag="ot")
                    nc.vector.tensor_tensor(ot[:], tmp[:], x1_t[:], OP.add)
                    nc.scalar.dma_start(out_d[t * 128:(t + 1) * 128,
                                              nn * 512:(nn + 1) * 512], ot[:])
        actq_pool.release()
        cs_pool.release()
        perm.release()

    nc.compile()
    return nc


_NC_CACHE = None


def _get_nc():
    global _NC_CACHE
    if _NC_CACHE is None:
        _NC_CACHE = build_program()
    return _NC_CACHE


def _ternarize(w):
    """Reference _weight_quant: returns (ternary float {-1,0,1}, dqw scale)."""
    w = np.asarray(w, np.float32)
    m = np.maximum(np.mean(np.abs(w), dtype=np.float32), np.float32(1e-5))
    scale = np.float32(1.0) / m
    tern = np.clip(np.round(w * scale), -1.0, 1.0).astype(np.float32)
    return tern, float(m)


def _strip_layout(w_t, n_strips, nk):
    """[in_feats, out_feats] -> [n_strips, 128, nk, 512] (strip s covers out
    cols s*512..). w_t is the transposed weight [in, out]."""
    infeat, outfeat = w_t.shape
    assert infeat == nk * 128 and outfeat == n_strips * 512
    # [nk, 128, n_strips, 512] -> [n_strips, 128, nk, 512]
    v = w_t.reshape(nk, 128, n_strips, 512)
    return np.ascontiguousarray(v.transpose(2, 1, 0, 3))


def _host_inputs(x, attn_norm_w, ffn_norm_w, qkv_w, o_w, ffn_up_w, ffn_down_w):
    x = np.ascontiguousarray(np.asarray(x, np.float32))
    anw = np.ascontiguousarray(np.asarray(attn_norm_w, np.float32))
    fnw = np.ascontiguousarray(np.asarray(ffn_norm_w, np.float32))

    tern_qkv, dqw_qkv = _ternarize(qkv_w)
    tern_o, dqw_o = _ternarize(o_w)
    tern_dn, dqw_dn = _ternarize(ffn_down_w)

    wqkv_sh = _strip_layout(tern_qkv.T, 12, NK).astype(ml_dtypes.bfloat16)
    wo_sh = _strip_layout(tern_o.T, 4, NK).astype(ml_dtypes.bfloat16)
    wup_sh = _strip_layout(np.asarray(ffn_up_w, np.float32).T, 16, NK) \
        .astype(ml_dtypes.bfloat16)
    wdn_sh = _strip_layout(tern_dn.T, 4, NKI).astype(ml_dtypes.bfloat16) \
        .reshape(4, 128, 2, 16, 512).transpose(0, 2, 1, 3, 4)
    wdn_sh = np.ascontiguousarray(wdn_sh)

    consts = np.zeros(8, np.float32)
    consts[0] = dqw_qkv * dqw_qkv * (HD ** -0.5)
    consts[1] = dqw_qkv * dqw_o
    consts[2] = dqw_dn

    # rope tables: cs[p, 0/1, t, 4*64] bf16, replicated x4 for the 4-head strips
    inv = 1.0 / (THETA ** (np.arange(0, HD, 2, dtype=np.float32) / HD))
    tpos = np.arange(S, dtype=np.float32)
    fr = np.outer(tpos, inv)                     # [S, 64]
    cos = np.tile(np.cos(fr), (1, 4))            # [S, 256]
    sin = np.tile(np.sin(fr), (1, 4))

    in_maps = []
    for c in range(NCORES):
        b = c // 4
        t0 = (c % 4) * R
        # [R, 256] -> [NT, 128, 256] -> [128, NT, 256]
        cs = np.stack([cos[t0:t0 + R].reshape(NT, 128, 256).transpose(1, 0, 2),
                       sin[t0:t0 + R].reshape(NT, 128, 256).transpose(1, 0, 2)],
                      axis=1)                    # [128, 2, NT, 256]
        in_maps.append({
            "x_sh": np.ascontiguousarray(x[b, t0:t0 + R, :]),
            "cs_sh": np.ascontiguousarray(cs).astype(ml_dtypes.bfloat16),
            "attn_norm_w": anw, "ffn_norm_w": fnw, "consts": consts,
            "wqkv_sh": wqkv_sh, "wo_sh": wo_sh, "wup_sh": wup_sh, "wdn_sh": wdn_sh,
        })
    return in_maps


def run(trace=False, **inputs):
    nc = _get_nc()
    in_maps = _host_inputs(**inputs)
    res = run_bass_kernel_spmd(nc, in_maps, list(range(NCORES)), trace=trace)
    out = np.empty((B, S, H), np.float32)
    for c in range(NCORES):
        b = c // 4
        t0 = (c % 4) * R
        out[b, t0:t0 + R, :] = res.results[c]["out_sh"]
    return out, res


def kernel(**inputs):
    out, _ = run(trace=False, **inputs)
    return out


# revision 50
# speedup vs baseline: 1.2153x; 1.0097x over previous
"""Trainium2 Bass kernel for nn_BlockAttentionResidual (block attention + BitNet-style quantized MLP).

Sharding: sequence-block data parallelism, zero collectives. Block attention is
independent per 512-token block, so each of the 8 cores owns 1024 contiguous
tokens (2 blocks) of one batch element and runs the whole layer on them.
  core c -> batch c//4, tokens [(c%4)*1024, (c%4+1)*1024)

Weights are static parameters: ternarization (per-tensor mean|w| scale, exact
reference semantics) is host-side preprocessing; the ternary {-1,0,1} values are
exact in bf16.  Per-tensor dequant scalars fold into three constants shipped as
a tiny input tensor (exp-scale for attention, psum-eviction scales for o_proj /
ffn_down), so the device does no dequant bookkeeping: activations are
quantize-dequantized in one fused ACT+DVE pair per tile and all matmuls run on
bf16 operands with fp32 PSUM accumulation.

All DRAM layouts are pre-tiled host-side so every DMA line is >=1KB contiguous
per partition (the previous kernel was DMA-descriptor-bound: 600k descriptors
averaging 800B).
"""

import numpy as np
import ml_dtypes

import concourse.bass as bass
import concourse.mybir as mybir
import concourse.tile as tile
from concourse import bacc
from concourse.bass_utils import run_bass_kernel_spmd

F32 = mybir.dt.float32
BF16 = mybir.dt.bfloat16
AX = mybir.AxisListType
OP = mybir.AluOpType
ACTF = mybir.ActivationFunctionType

# model dims
H = 2048
NH = 16
HD = 128
NB = 8
INTER = 4096        # 2*H
EPS = 1e-5
THETA = 10000.0
B, S = 2, 4096
BT = 512            # tokens per attention block
NCORES = 8
R = 1024            # tokens per core
NT = R // 128       # 8 token tiles per core
NK = H // 128       # 16 k-tiles of the hidden dim
NKI = INTER // 128  # 32 k-tiles of the intermediate dim
MAGIC = np.float32(1.5 * 2 ** 23)   # fp32 round-to-nearest-even magic


def _quant_pair(nc, pool, src_ap, ncols, amax_ap, s_store, dq_store, out_bf,
                magic_ap, tag):
    """Quantize-dequantize src_ap [128, ncols] onto the int8 grid:
    out_bf = round(src*127/amax) * amax/127 in bf16.  amax_ap: [128,1] f32."""
    amc = pool.tile([128, 1], F32, tag=f"amc_{tag}")
    nc.vector.tensor_scalar_max(amc[:], amax_ap, 1e-5)
    rec = pool.tile([128, 1], F32, tag=f"rec_{tag}")
    nc.vector.reciprocal(rec[:], amc[:])
    nc.vector.tensor_scalar_mul(s_store, rec[:], 127.0)
    nc.vector.tensor_scalar_mul(dq_store, amc[:], 1.0 / 127.0)
    mg = pool.tile([128, ncols], F32, tag=f"mg_{tag}")
    nc.scalar.activation(mg[:], src_ap, ACTF.Identity, bias=magic_ap, scale=s_store)
    nc.vector.tensor_scalar(out_bf, mg[:], float(MAGIC), dq_store, OP.subtract,
                            OP.mult)


def build_program():
    nc = bacc.Bacc(None, target_bir_lowering=False)

    # ---- I/O (all layouts pre-tiled on host) ----
    x_in = nc.declare_dram_parameter("x_sh", [R, H], F32, isOutput=False)
    cs_in = nc.declare_dram_parameter("cs_sh", [128, 2, NT, 256], BF16, isOutput=False)
    anw_in = nc.declare_dram_parameter("attn_norm_w", [H], F32, isOutput=False)
    fnw_in = nc.declare_dram_parameter("ffn_norm_w", [H], F32, isOutput=False)
    consts_in = nc.declare_dram_parameter("consts", [8], F32, isOutput=False)
    # weight strips: [strip, 128 kpart, n_ktiles, 512 outcols] bf16
    wqkv_in = nc.declare_dram_parameter("wqkv_sh", [12, 128, NK, 512], BF16, isOutput=False)
    wo_in = nc.declare_dram_parameter("wo_sh", [4, 128, NK, 512], BF16, isOutput=False)
    wup_in = nc.declare_dram_parameter("wup_sh", [16, 128, NK, 512], BF16, isOutput=False)
    # ffn_down strips split in two k-halves: [strip, half, 128, 16, 512]
    wdn_in = nc.declare_dram_parameter("wdn_sh", [4, 2, 128, 16, 512], BF16, isOutput=False)
    out_d = nc.declare_dram_parameter("out_sh", [R, H], F32, isOutput=True)

    # ---- internal DRAM scratch ----
    x1_d = nc.dram_tensor("x1_d", [NT, 128, H], F32)
    act_d = nc.dram_tensor("act_d", [NT, 128, INTER], BF16)      # silu(g)*v rows

    with tile.TileContext(nc) as tc:
        perm = tc.alloc_tile_pool(name="perm", bufs=1)
        magic_t = perm.tile([128, 1], F32)
        nc.vector.memset(magic_t[:], float(MAGIC))
        magic_ap = magic_t[:]
        consts_b = perm.tile([128, 8], F32)
        ap0 = consts_in[:]
        nc.gpsimd.dma_start(out=consts_b[:], in_=bass.AP(
            tensor=ap0.tensor, offset=ap0.offset, ap=[[0, 128]] + list(ap0.ap)))
        c_att = consts_b[:, 0:1]   # dqw_qkv^2 * HD^-0.5
        c_o = consts_b[:, 1:2]     # dqw_qkv * dqw_o
        c_dn = consts_b[:, 2:3]    # dqw_dn
        # per-token quant scales (s = 127/amax, dq = amax/127)
        s1 = perm.tile([128, NT], F32)
        dq1 = perm.tile([128, NT], F32)
        sc = perm.tile([128, NT], F32)
        dqc = perm.tile([128, NT], F32)
        sa = perm.tile([128, NT], F32)
        dqa = perm.tile([128, NT], F32)
        amax_av = perm.tile([128, NT], F32)
        nc.vector.memset(amax_av[:], 0.0)

        ssq_parts = perm.tile([128, NT, 4], F32)

        cs_pool = tc.alloc_tile_pool(name="cs_pool", bufs=1)
        cs_b = cs_pool.tile([128, 2, NT, 256], BF16)
        nc.gpsimd.dma_start(cs_b[:], cs_in[:])

        ctx_pool = tc.alloc_tile_pool(name="ctx_pool", bufs=1)
        ctx_sb = ctx_pool.tile([128, NT, NH, 128], BF16)
        slot1 = tc.alloc_tile_pool(name="slot1", bufs=1)
        xqT = slot1.tile([128, NT, NK, 128], BF16)

        # qkv weight pool opened early so the first strips prefetch during P1
        qkwpool = tc.alloc_tile_pool(name="qkwpool", bufs=2)
        wst_pre = []
        for kind in range(2):
            wst = qkwpool.tile([128, NK, 512], BF16, tag="w_qkv")
            nc.gpsimd.dma_start(wst[:], wqkv_in[4 * kind, :, :, :])
            wst_pre.append(wst)

        # ---------- P1: attn rmsnorm + act-qdq + transpose ----------
        with tc.tile_pool(name="npool", bufs=2) as npool, \
             tc.tile_pool(name="nwpool", bufs=1) as nwpool:
            anw_b = nwpool.tile([128, H], F32, tag="normw")
            ap1 = anw_in[:]
            nc.gpsimd.dma_start(out=anw_b[:], in_=bass.AP(
                tensor=ap1.tensor, offset=ap1.offset, ap=[[0, 128]] + list(ap1.ap)))
            for t in range(NT):
                xt = npool.tile([128, H], F32, tag="xt")
                nc.scalar.dma_start(xt[:], x_in[t * 128:(t + 1) * 128, :])
                ssq = npool.tile([128, 1], F32, tag="ssq")
                junk = npool.tile([128, H], BF16, tag="njunk")
                nc.scalar.activation(junk[:], xt[:], ACTF.Square, accum_out=ssq[:])
                msq = npool.tile([128, 1], F32, tag="msq")
                nc.vector.tensor_scalar(msq[:], ssq[:], 1.0 / H, EPS, OP.mult, OP.add)
                sd = npool.tile([128, 1], F32, tag="sd")
                nc.scalar.activation(sd[:], msq[:], ACTF.Sqrt)
                rstd = npool.tile([128, 1], F32, tag="rstd")
                nc.vector.reciprocal(rstd[:], sd[:])
                h_t = npool.tile([128, H], F32, tag="h_t")
                nc.vector.tensor_scalar_mul(h_t[:], xt[:], rstd[:])
                nc.gpsimd.tensor_tensor(h_t[:], h_t[:], anw_b[:], OP.mult)
                amax = npool.tile([128, 1], F32, tag="amax_n1")
                nc.vector.tensor_reduce(amax[:], h_t[:], AX.X, OP.max,
                                        apply_absolute_value=True)
                xq = npool.tile([128, H], BF16, tag="xq")
                _quant_pair(nc, npool, h_t[:], H, amax[:], s1[:, t:t + 1],
                            dq1[:, t:t + 1], xq[:], magic_ap, "n1")
                nc.sync.dma_start_transpose(xqT[:, t, :, :], xq[:])

        # ---------- P2+P3: qkv matmul + rope interleaved with block attention ----------
        # strips ordered q_g,k_g,v_g per 4-head group g; attention for group g
        # runs right after its three strips, overlapping later groups' matmuls.
        with tc.tile_pool(name="ropool", bufs=3) as ropool, \
             tc.tile_pool(name="qkpool", bufs=2) as qkpool, \
             tc.tile_pool(name="apool", bufs=2) as apool, \
             tc.tile_pool(name="qkv_psum", bufs=3, space="PSUM") as psum_mm, \
             tc.tile_pool(name="at_psum", bufs=3, space="PSUM", side="right") as psum_at, \
             tc.tile_pool(name="ctx_psum", bufs=2, space="PSUM", side="right") as psum_cx:
            for g in range(4):
                qTg = qkpool.tile([128, NT, 4, 128], BF16, tag="qTg")
                kTg = qkpool.tile([128, NT, 4, 128], BF16, tag="kTg")
                v_g = qkpool.tile([128, NT, 4, 132], BF16, tag="vg")
                nc.vector.memset(v_g[:, :, :, 128:129], 1.0)
                for kind in range(3):
                    src = g + 4 * kind
                    if g == 0 and kind < 2:
                        wst = wst_pre[kind]
                    else:
                        wst = qkwpool.tile([128, NK, 512], BF16, tag="w_qkv")
                        nc.gpsimd.dma_start(wst[:], wqkv_in[src, :, :, :])
                    for t in range(NT):
                        ps = psum_mm.tile([128, 512], F32, tag="ps_qkv")
                        for kk in range(NK):
                            nc.tensor.matmul(ps[:], xqT[:, t, kk, :], wst[:, kk, :],
                                             start=(kk == 0), stop=(kk == NK - 1))
                        if kind == 2:
                            nc.scalar.copy(v_g[:, t, :, 0:128],
                                           ps[:].rearrange("p (c f) -> p c f", c=4))
                        else:
                            qsc = ropool.tile([128, 4, 128], BF16, tag="qsc")
                            nc.scalar.copy(qsc[:], ps[:].rearrange("p (c f) -> p c f", c=4))
                            p1, p2 = qsc[:, :, 0:64], qsc[:, :, 64:128]
                            cosd = cs_b[:, 0, t, :].rearrange("p (c f) -> p c f", c=4)
                            sind = cs_b[:, 1, t, :].rearrange("p (c f) -> p c f", c=4)
                            t1 = ropool.tile([128, 4, 64], F32, tag="rt1")
                            t2 = ropool.tile([128, 4, 64], F32, tag="rt2")
                            rot = ropool.tile([128, 4, 128], BF16, tag="rot")
                            nc.vector.tensor_tensor(t1[:], p1, cosd, OP.mult)
                            nc.gpsimd.tensor_tensor(t2[:], p2, sind, OP.mult)
                            nc.vector.tensor_tensor(rot[:, :, 0:64], t1[:], t2[:], OP.subtract)
                            nc.vector.tensor_tensor(t1[:], p2, cosd, OP.mult)
                            nc.gpsimd.tensor_tensor(t2[:], p1, sind, OP.mult)
                            nc.vector.tensor_tensor(rot[:, :, 64:128], t1[:], t2[:], OP.add)
                            dst = qTg if kind == 0 else kTg
                            nc.sync.dma_start_transpose(
                                dst[:, t, :, :], rot[:].rearrange("p c f -> p (c f)"))
                for blk in range(2):
                    for hl in range(4):
                        expT = [None] * 4
                        for kt in range(4):
                            qn = 512 - kt * 128
                            pss = psum_at.tile([128, 512], F32, tag="ps_sc")
                            nc.tensor.matmul(
                                pss[:, 0:qn],
                                kTg[:, blk * 4 + kt, hl, :],
                                qTg[:, blk * 4 + kt:(blk + 1) * 4, hl, :],
                                start=True, stop=True)
                            ex = apool.tile([128, 512], BF16, tag=f"expT{kt}")
                            nc.scalar.activation(ex[:, 0:qn], pss[:, 0:qn], ACTF.Exp,
                                                 scale=c_att)
                            nc.gpsimd.affine_select(
                                out=ex[:, 0:128], in_=ex[:, 0:128],
                                compare_op=OP.is_ge, fill=0.0,
                                base=0, pattern=[[1, 128]], channel_multiplier=-1)
                            expT[kt] = ex
                        for qt in range(4):
                            psc = psum_cx.tile([128, 132], F32, tag="ps_ctx")
                            for kt in range(qt + 1):
                                nc.tensor.matmul(psc[:, 0:129],
                                                 expT[kt][:, (qt - kt) * 128:(qt - kt) * 128 + 128],
                                                 v_g[:, blk * 4 + kt, hl, 0:129],
                                                 start=(kt == 0), stop=(kt == qt))
                            rl = apool.tile([128, 1], F32, tag="rl")
                            nc.vector.reciprocal(rl[:], psc[:, 128:129])
                            nc.vector.tensor_scalar_mul(
                                ctx_sb[:, blk * 4 + qt, 4 * g + hl, :],
                                psc[:, 0:128], rl[:])
        qkwpool.release()
        slot1.release()

        # ---------- P4: ctx act-qdq + transpose ----------
        ctxq_pool = tc.alloc_tile_pool(name="ctxq_pool", bufs=1, side="right")
        ctxqT = ctxq_pool.tile([128, NT, NK, 128], BF16)
        with tc.tile_pool(name="cqpool", bufs=2) as cqpool:
            for t in range(NT):
                src = ctx_sb[:, t, :, :].rearrange("p c f -> p (c f)")
                amax = cqpool.tile([128, 1], F32, tag="amax_cq")
                nc.vector.tensor_reduce(amax[:], src, AX.X, OP.max,
                                        apply_absolute_value=True)
                cq = cqpool.tile([128, H], BF16, tag="cq")
                _quant_pair(nc, cqpool, src, H, amax[:], sc[:, t:t + 1],
                            dqc[:, t:t + 1], cq[:], magic_ap, "cq")
                nc.sync.dma_start_transpose(ctxqT[:, t, :, :], cq[:])
        ctx_pool.release()

        # ---------- P5: o_proj + residual -> x1_d (+ ffn-norm ssq fused) ----------
        # ---------- P6: ffn rmsnorm (slim) + transpose ----------
        hnT_pool = tc.alloc_tile_pool(name="hnT_pool", bufs=1)
        hnT = hnT_pool.tile([128, NT, NK, 128], BF16)
        upool = tc.alloc_tile_pool(name="upool", bufs=2)
        with tc.tile_pool(name="opool", bufs=3) as opool, \
             tc.tile_pool(name="owpool", bufs=2) as owpool, \
             tc.tile_pool(name="n2pool", bufs=2) as n2pool, \
             tc.tile_pool(name="n2wpool", bufs=1) as n2wpool, \
             tc.tile_pool(name="o_psum", bufs=3, space="PSUM") as psum_o:
            fnw_b = n2wpool.tile([128, H], F32, tag="normw2")
            ap2 = fnw_in[:]
            nc.gpsimd.dma_start(out=fnw_b[:], in_=bass.AP(
                tensor=ap2.tensor, offset=ap2.offset, ap=[[0, 128]] + list(ap2.ap)))
            for nn in range(4):
                wst = owpool.tile([128, NK, 512], BF16, tag="wo_st")
                nc.gpsimd.dma_start(wst[:], wo_in[nn, :, :, :])
                for t in range(NT):
                    ps = psum_o.tile([128, 512], F32, tag="ps_o")
                    for kk in range(NK):
                        nc.tensor.matmul(ps[:], ctxqT[:, t, kk, :], wst[:, kk, :],
                                         start=(kk == 0), stop=(kk == NK - 1))
                    xs = opool.tile([128, 512], F32, tag="xs")
                    nc.scalar.dma_start(xs[:], x_in[t * 128:(t + 1) * 128,
                                                    nn * 512:(nn + 1) * 512])
                    tmp = opool.tile([128, 512], F32, tag="o_tmp")
                    nc.scalar.activation(tmp[:], ps[:], ACTF.Identity, scale=c_o)
                    x1s = opool.tile([128, 512], F32, tag="x1s")
                    nc.vector.tensor_tensor(x1s[:], tmp[:], xs[:], OP.add)
                    sqt = opool.tile([128, 512], F32, tag="o_sq")
                    nc.vector.tensor_tensor(sqt[:], x1s[:], x1s[:], OP.mult)
                    nc.vector.tensor_reduce(ssq_parts[:, t, nn:nn + 1], sqt[:],
                                            AX.X, OP.add)
                    nc.scalar.dma_start(x1_d[t, :, nn * 512:(nn + 1) * 512], x1s[:])
            up_pre = []
            for j in range(2):
                w = upool.tile([128, NK, 512], BF16, tag="w_up")
                nc.gpsimd.dma_start(w[:], wup_in[8 * j, :, :, :])
                up_pre.append(w)
            for t in range(NT):
                x1_t = n2pool.tile([128, H], F32, tag="x1n")
                nc.sync.dma_start(x1_t[:], x1_d[t, :, :])
                ssq = n2pool.tile([128, 1], F32, tag="ssq2")
                nc.vector.tensor_reduce(ssq[:], ssq_parts[:, t, :], AX.X, OP.add)
                msq = n2pool.tile([128, 1], F32, tag="msq2")
                nc.vector.tensor_scalar(msq[:], ssq[:], 1.0 / H, EPS, OP.mult, OP.add)
                sd = n2pool.tile([128, 1], F32, tag="sd2")
                nc.scalar.activation(sd[:], msq[:], ACTF.Sqrt)
                rstd = n2pool.tile([128, 1], F32, tag="rstd2")
                nc.vector.reciprocal(rstd[:], sd[:])
                hn_t = n2pool.tile([128, H], BF16, tag="hn_t")
                nc.vector.tensor_scalar_mul(hn_t[:], x1_t[:], rstd[:])
                nc.gpsimd.tensor_tensor(hn_t[:], hn_t[:], fnw_b[:], OP.mult)
                nc.sync.dma_start_transpose(hnT[:, t, :, :], hn_t[:])
        ctxq_pool.release()

        # ---------- P7: ffn up + silu*val -> act_d ----------
        with tc.tile_pool(name="fpool", bufs=3) as fpool, \
             tc.tile_pool(name="up_psum", bufs=3, space="PSUM", side="right") as psum_up, \
             tc.tile_pool(name="upv_psum", bufs=3, space="PSUM", side="right") as psum_upv:
            for i in range(8):   # paired gate/val strips of 512
                if i == 0:
                    wgr, wvr = up_pre
                else:
                    wgr = upool.tile([128, NK, 512], BF16, tag="w_up")
                    nc.gpsimd.dma_start(wgr[:], wup_in[i, :, :, :])
                    wvr = upool.tile([128, NK, 512], BF16, tag="w_up")
                    nc.gpsimd.dma_start(wvr[:], wup_in[8 + i, :, :, :])
                for t in range(NT):
                    psg = psum_up.tile([128, 512], F32, tag="ps_g")
                    for kk in range(NK):
                        nc.tensor.matmul(psg[:], hnT[:, t, kk, :], wgr[:, kk, :],
                                         start=(kk == 0), stop=(kk == NK - 1))
                    psv = psum_upv.tile([128, 512], F32, tag="ps_v")
                    for kk in range(NK):
                        nc.tensor.matmul(psv[:], hnT[:, t, kk, :], wvr[:, kk, :],
                                         start=(kk == 0), stop=(kk == NK - 1))
                    sgm = fpool.tile([128, 512], F32, tag="sgm")
                    nc.scalar.activation(sgm[:], psg[:], ACTF.Sigmoid)
                    sg = fpool.tile([128, 512], F32, tag="sg")
                    nc.vector.tensor_tensor(sg[:], sgm[:], psg[:], OP.mult)
                    av = fpool.tile([128, 512], BF16, tag="av")
                    nc.vector.tensor_tensor(av[:], sg[:], psv[:], OP.mult)
                    nc.sync.dma_start(act_d[t, :, i * 512:(i + 1) * 512], av[:])
                    rmax = fpool.tile([128, 1], F32, tag="rmax")
                    nc.vector.tensor_reduce(rmax[:], av[:], AX.X, OP.max,
                                            apply_absolute_value=True)
                    nc.vector.tensor_tensor(amax_av[:, t:t + 1], amax_av[:, t:t + 1],
                                            rmax[:], OP.max)

        upool.release()
        hnT_pool.release()

        # ---------- P8: ffn act-qdq + transpose -> actqT (SBUF) ----------
        # ---------- P9: ffn down + residual -> out ----------
        actq_pool = tc.alloc_tile_pool(name="actq_pool", bufs=1)
        actqT = actq_pool.tile([128, NT, NKI, 128], BF16)
        with tc.tile_pool(name="aqpool", bufs=2) as aqpool, \
             tc.tile_pool(name="dpool", bufs=2) as dpool, \
             tc.tile_pool(name="dopool", bufs=3) as dopool, \
             tc.tile_pool(name="dn_psum", bufs=4, space="PSUM") as psum_dn:
            # prefetch first down-weight strip ahead of the quant pass
            wdn_tiles = {}
            for half in range(2):
                w = dpool.tile([128, 16, 512], BF16, tag=f"w_dn_{half}")
                nc.gpsimd.dma_start(w[:], wdn_in[0, half, :, :, :])
                wdn_tiles[(0, half)] = w
            for t in range(NT):
                amc = aqpool.tile([128, 1], F32, tag="amc_a")
                nc.vector.tensor_scalar_max(amc[:], amax_av[:, t:t + 1], 1e-5)
                rec = aqpool.tile([128, 1], F32, tag="rec_a")
                nc.vector.reciprocal(rec[:], amc[:])
                nc.vector.tensor_scalar_mul(sa[:, t:t + 1], rec[:], 127.0)
                nc.vector.tensor_scalar_mul(dqa[:, t:t + 1], amc[:], 1.0 / 127.0)
                for hf in range(4):
                    at = aqpool.tile([128, 1024], BF16, tag="at")
                    nc.scalar.dma_start(at[:], act_d[t, :, hf * 1024:(hf + 1) * 1024])
                    aq = aqpool.tile([128, 1024], BF16, tag="aq")
                    if hf % 2 == 0:
                        mg = aqpool.tile([128, 1024], F32, tag="mg_a")
                        nc.scalar.activation(mg[:], at[:], ACTF.Identity,
                                             bias=magic_ap, scale=sa[:, t:t + 1])
                        nc.vector.tensor_scalar(aq[:], mg[:], float(MAGIC),
                                                dqa[:, t:t + 1], OP.subtract, OP.mult)
                    else:
                        mg = aqpool.tile([128, 1024], F32, tag="mg_b")
                        nc.vector.tensor_scalar(mg[:], at[:], sa[:, t:t + 1],
                                                float(MAGIC), OP.mult, OP.add)
                        nc.vector.tensor_scalar(aq[:], mg[:], float(MAGIC),
                                                dqa[:, t:t + 1], OP.subtract, OP.mult)
                    nc.sync.dma_start_transpose(actqT[:, t, hf * 8:(hf + 1) * 8, :],
                                                aq[:])
            for nn in range(4):
                if nn > 0:
                    for half in range(2):
                        w = dpool.tile([128, 16, 512], BF16, tag=f"w_dn_{half}")
                        nc.gpsimd.dma_start(w[:], wdn_in[nn, half, :, :, :])
                        wdn_tiles[(nn, half)] = w
                wsa = wdn_tiles[(nn, 0)]
                wsb = wdn_tiles[(nn, 1)]
                for t in range(NT):
                    ps = psum_dn.tile([128, 512], F32, tag="ps_dn")
                    for kk in range(NKI):
                        w = wsa[:, kk, :] if kk < 16 else wsb[:, kk - 16, :]
                        nc.tensor.matmul(ps[:], actqT[:, t, kk, :], w,
                                         start=(kk == 0), stop=(kk == NKI - 1))
                    x1_t = dopool.tile([128, 512], F32, tag="x1_re")
                    nc.scalar.dma_start(x1_t[:], x1_d[t, :, nn * 512:(nn + 1) * 512])
                    tmp = dopool.tile([128, 512], F32, tag="d_tmp")
                    nc.scalar.activation(tmp[:], ps[:], ACTF.Identity, scale=c_dn)
                    ot = dopool.tile([128, 512], F32, tag="ot")
                    nc.vector.tensor_tensor(ot[:], tmp[:], x1_t[:], OP.add)
                    nc.scalar.dma_start(out_d[t * 128:(t + 1) * 128,
                                              nn * 512:(nn + 1) * 512], ot[:])
        actq_pool.release()
        cs_pool.release()
        perm.release()

    nc.compile()
    return nc


_NC_CACHE = None


def _get_nc():
    global _NC_CACHE
    if _NC_CACHE is None:
        _NC_CACHE = build_program()
    return _NC_CACHE


def _ternarize(w):
    """Reference _weight_quant: returns (ternary float {-1,0,1}, dqw scale)."""
    w = np.asarray(w, np.float32)
    m = np.maximum(np.mean(np.abs(w), dtype=np.float32), np.float32(1e-5))
    scale = np.float32(1.0) / m
    tern = np.clip(np.round(w * scale), -1.0, 1.0).astype(np.float32)
    return tern, float(m)


def _strip_layout(w_t, n_strips, nk):
    """[in_feats, out_feats] -> [n_strips, 128, nk, 512] (strip s covers out
    cols s*512..). w_t is the transposed weight [in, out]."""
    infeat, outfeat = w_t.shape
    assert infeat == nk * 128 and outfeat == n_strips * 512
    # [nk, 128, n_strips, 512] -> [n_strips, 128, nk, 512]
    v = w_t.reshape(nk, 128, n_strips, 512)
    return np.ascontiguousarray(v.transpose(2, 1, 0, 3))


def _host_inputs(x, attn_norm_w, ffn_norm_w, qkv_w, o_w, ffn_up_w, ffn_down_w):
    x = np.ascontiguousarray(np.asarray(x, np.float32))
    anw = np.ascontiguousarray(np.asarray(attn_norm_w, np.float32))
    fnw = np.ascontiguousarray(np.asarray(ffn_norm_w, np.float32))

    tern_qkv, dqw_qkv = _ternarize(qkv_w)
    tern_o, dqw_o = _ternarize(o_w)
    tern_dn, dqw_dn = _ternarize(ffn_down_w)

    wqkv_sh = _strip_layout(tern_qkv.T, 12, NK).astype(ml_dtypes.bfloat16)
    wo_sh = _strip_layout(tern_o.T, 4, NK).astype(ml_dtypes.bfloat16)
    wup_sh = _strip_layout(np.asarray(ffn_up_w, np.float32).T, 16, NK) \
        .astype(ml_dtypes.bfloat16)
    wdn_sh = _strip_layout(tern_dn.T, 4, NKI).astype(ml_dtypes.bfloat16) \
        .reshape(4, 128, 2, 16, 512).transpose(0, 2, 1, 3, 4)
    wdn_sh = np.ascontiguousarray(wdn_sh)

    consts = np.zeros(8, np.float32)
    consts[0] = dqw_qkv * dqw_qkv * (HD ** -0.5)
    consts[1] = dqw_qkv * dqw_o
    consts[2] = dqw_dn

    # rope tables: cs[p, 0/1, t, 4*64] bf16, replicated x4 for the 4-head strips
    inv = 1.0 / (THETA ** (np.arange(0, HD, 2, dtype=np.float32) / HD))
    tpos = np.arange(S, dtype=np.float32)
    fr = np.outer(tpos, inv)                     # [S, 64]
    cos = np.tile(np.cos(fr), (1, 4))            # [S, 256]
    sin = np.tile(np.sin(fr), (1, 4))

    in_maps = []
    for c in range(NCORES):
        b = c // 4
        t0 = (c % 4) * R
        # [R, 256] -> [NT, 128, 256] -> [128, NT, 256]
        cs = np.stack([cos[t0:t0 + R].reshape(NT, 128, 256).transpose(1, 0, 2),
                       sin[t0:t0 + R].reshape(NT, 128, 256).transpose(1, 0, 2)],
                      axis=1)                    # [128, 2, NT, 256]
        in_maps.append({
            "x_sh": np.ascontiguousarray(x[b, t0:t0 + R, :]),
            "cs_sh": np.ascontiguousarray(cs).astype(ml_dtypes.bfloat16),
            "attn_norm_w": anw, "ffn_norm_w": fnw, "consts": consts,
            "wqkv_sh": wqkv_sh, "wo_sh": wo_sh, "wup_sh": wup_sh, "wdn_sh": wdn_sh,
        })
    return in_maps


def run(trace=False, **inputs):
    nc = _get_nc()
    in_maps = _host_inputs(**inputs)
    res = run_bass_kernel_spmd(nc, in_maps, list(range(NCORES)), trace=trace)
    out = np.empty((B, S, H), np.float32)
    for c in range(NCORES):
        b = c // 4
        t0 = (c % 4) * R
        out[b, t0:t0 + R, :] = res.results[c]["out_sh"]
    return out, res


def kernel(**inputs):
    out, _ = run(trace=False, **inputs)
    return out


# revision 52
# speedup vs baseline: 1.2721x; 1.0467x over previous
"""Trainium2 Bass kernel for nn_BlockAttentionResidual (block attention + BitNet-style quantized MLP).

Sharding: sequence-block data parallelism, zero collectives. Block attention is
independent per 512-token block, so each of the 8 cores owns 1024 contiguous
tokens (2 blocks) of one batch element and runs the whole layer on them.
  core c -> batch c//4, tokens [(c%4)*1024, (c%4+1)*1024)

Weights are static parameters: ternarization (per-tensor mean|w| scale, exact
reference semantics) is host-side preprocessing; the ternary {-1,0,1} values are
exact in bf16.  Per-tensor dequant scalars fold into three constants shipped as
a tiny input tensor (exp-scale for attention, psum-eviction scales for o_proj /
ffn_down), so the device does no dequant bookkeeping: activations are
quantize-dequantized in one fused ACT+DVE pair per tile and all matmuls run on
bf16 operands with fp32 PSUM accumulation.

All DRAM layouts are pre-tiled host-side so every DMA line is >=1KB contiguous
per partition (the previous kernel was DMA-descriptor-bound: 600k descriptors
averaging 800B).
"""

import numpy as np
import ml_dtypes

import concourse.bass as bass
import concourse.mybir as mybir
import concourse.tile as tile
from concourse import bacc
from concourse.bass_utils import run_bass_kernel_spmd

F32 = mybir.dt.float32
BF16 = mybir.dt.bfloat16
AX = mybir.AxisListType
OP = mybir.AluOpType
ACTF = mybir.ActivationFunctionType

# model dims
H = 2048
NH = 16
HD = 128
NB = 8
INTER = 4096        # 2*H
EPS = 1e-5
THETA = 10000.0
B, S = 2, 4096
BT = 512            # tokens per attention block
NCORES = 8
R = 1024            # tokens per core
NT = R // 128       # 8 token tiles per core
NK = H // 128       # 16 k-tiles of the hidden dim
NKI = INTER // 128  # 32 k-tiles of the intermediate dim
MAGIC = np.float32(1.5 * 2 ** 23)   # fp32 round-to-nearest-even magic


def _quant_pair(nc, pool, src_ap, ncols, amax_ap, s_store, dq_store, out_bf,
                magic_ap, tag):
    """Quantize-dequantize src_ap [128, ncols] onto the int8 grid:
    out_bf = round(src*127/amax) * amax/127 in bf16.  amax_ap: [128,1] f32."""
    amc = pool.tile([128, 1], F32, tag=f"amc_{tag}")
    nc.vector.tensor_scalar_max(amc[:], amax_ap, 1e-5)
    rec = pool.tile([128, 1], F32, tag=f"rec_{tag}")
    nc.vector.reciprocal(rec[:], amc[:])
    nc.vector.tensor_scalar_mul(s_store, rec[:], 127.0)
    nc.vector.tensor_scalar_mul(dq_store, amc[:], 1.0 / 127.0)
    mg = pool.tile([128, ncols], F32, tag=f"mg_{tag}")
    nc.scalar.activation(mg[:], src_ap, ACTF.Identity, bias=magic_ap, scale=s_store)
    nc.vector.tensor_scalar(out_bf, mg[:], float(MAGIC), dq_store, OP.subtract,
                            OP.mult)


def build_program():
    nc = bacc.Bacc(None, target_bir_lowering=False)

    # ---- I/O (all layouts pre-tiled on host) ----
    x_in = nc.declare_dram_parameter("x_sh", [R, H], F32, isOutput=False)
    cs_in = nc.declare_dram_parameter("cs_sh", [128, 2, NT, 256], BF16, isOutput=False)
    anw_in = nc.declare_dram_parameter("attn_norm_w", [H], F32, isOutput=False)
    fnw_in = nc.declare_dram_parameter("ffn_norm_w", [H], F32, isOutput=False)
    consts_in = nc.declare_dram_parameter("consts", [8], F32, isOutput=False)
    # weight strips: [strip, 128 kpart, n_ktiles, 512 outcols] bf16
    wqkv_in = nc.declare_dram_parameter("wqkv_sh", [12, 128, NK, 512], BF16, isOutput=False)
    wo_in = nc.declare_dram_parameter("wo_sh", [4, 128, NK, 512], BF16, isOutput=False)
    wup_in = nc.declare_dram_parameter("wup_sh", [16, 128, NK, 512], BF16, isOutput=False)
    # ffn_down strips split in two k-halves: [strip, half, 128, 16, 512]
    wdn_in = nc.declare_dram_parameter("wdn_sh", [4, 2, 128, 16, 512], BF16, isOutput=False)
    out_d = nc.declare_dram_parameter("out_sh", [R, H], F32, isOutput=True)

    # ---- internal DRAM scratch ----
    x1_d = nc.dram_tensor("x1_d", [NT, 128, H], F32)
    act_d = nc.dram_tensor("act_d", [NT, 128, INTER], BF16)      # silu(g)*v rows

    with tile.TileContext(nc) as tc:
        perm = tc.alloc_tile_pool(name="perm", bufs=1)
        magic_t = perm.tile([128, 1], F32)
        nc.vector.memset(magic_t[:], float(MAGIC))
        magic_ap = magic_t[:]
        consts_b = perm.tile([128, 8], F32)
        ap0 = consts_in[:]
        nc.gpsimd.dma_start(out=consts_b[:], in_=bass.AP(
            tensor=ap0.tensor, offset=ap0.offset, ap=[[0, 128]] + list(ap0.ap)))
        c_att = consts_b[:, 0:1]   # dqw_qkv^2 * HD^-0.5
        c_o = consts_b[:, 1:2]     # dqw_qkv * dqw_o
        c_dn = consts_b[:, 2:3]    # dqw_dn
        # per-token quant scales (s = 127/amax, dq = amax/127)
        s1 = perm.tile([128, NT], F32)
        dq1 = perm.tile([128, NT], F32)
        sc = perm.tile([128, NT], F32)
        dqc = perm.tile([128, NT], F32)
        sa = perm.tile([128, NT], F32)
        dqa = perm.tile([128, NT], F32)
        amax_av = perm.tile([128, NT], F32)
        nc.vector.memset(amax_av[:], 0.0)

        ssq_parts = perm.tile([128, NT, 4], F32)

        cs_pool = tc.alloc_tile_pool(name="cs_pool", bufs=1)
        cs_b = cs_pool.tile([128, 2, NT, 256], BF16)
        nc.gpsimd.dma_start(cs_b[:], cs_in[:])

        ctx_pool = tc.alloc_tile_pool(name="ctx_pool", bufs=1)
        ctx_sb = ctx_pool.tile([128, NT, NH, 128], BF16)
        slot1 = tc.alloc_tile_pool(name="slot1", bufs=1)
        xqT = slot1.tile([128, NT, NK, 128], BF16)

        # qkv weight pool opened early so the first strips prefetch during P1
        qkwpool = tc.alloc_tile_pool(name="qkwpool", bufs=2)
        wst_pre = []
        for kind in range(2):
            wst = qkwpool.tile([128, NK, 512], BF16, tag="w_qkv")
            nc.gpsimd.dma_start(wst[:], wqkv_in[4 * kind, :, :, :])
            wst_pre.append(wst)

        # ---------- P1: attn rmsnorm + act-qdq + transpose ----------
        with tc.tile_pool(name="npool", bufs=2) as npool, \
             tc.tile_pool(name="nwpool", bufs=1) as nwpool:
            anw_b = nwpool.tile([128, H], F32, tag="normw")
            ap1 = anw_in[:]
            nc.gpsimd.dma_start(out=anw_b[:], in_=bass.AP(
                tensor=ap1.tensor, offset=ap1.offset, ap=[[0, 128]] + list(ap1.ap)))
            for t in range(NT):
                xt = npool.tile([128, H], F32, tag="xt")
                nc.scalar.dma_start(xt[:], x_in[t * 128:(t + 1) * 128, :])
                ssq = npool.tile([128, 1], F32, tag="ssq")
                junk = npool.tile([128, H], BF16, tag="njunk")
                nc.scalar.activation(junk[:], xt[:], ACTF.Square, accum_out=ssq[:])
                msq = npool.tile([128, 1], F32, tag="msq")
                nc.vector.tensor_scalar(msq[:], ssq[:], 1.0 / H, EPS, OP.mult, OP.add)
                sd = npool.tile([128, 1], F32, tag="sd")
                nc.scalar.activation(sd[:], msq[:], ACTF.Sqrt)
                rstd = npool.tile([128, 1], F32, tag="rstd")
                nc.vector.reciprocal(rstd[:], sd[:])
                h_t = npool.tile([128, H], F32, tag="h_t")
                nc.vector.tensor_scalar_mul(h_t[:], xt[:], rstd[:])
                nc.gpsimd.tensor_tensor(h_t[:], h_t[:], anw_b[:], OP.mult)
                amax = npool.tile([128, 1], F32, tag="amax_n1")
                nc.vector.tensor_reduce(amax[:], h_t[:], AX.X, OP.max,
                                        apply_absolute_value=True)
                xq = npool.tile([128, H], BF16, tag="xq")
                _quant_pair(nc, npool, h_t[:], H, amax[:], s1[:, t:t + 1],
                            dq1[:, t:t + 1], xq[:], magic_ap, "n1")
                nc.sync.dma_start_transpose(xqT[:, t, :, :], xq[:])

        # ---------- P2+P3: qkv matmul + rope interleaved with block attention ----------
        # strips ordered q_g,k_g,v_g per 4-head group g; attention for group g
        # runs right after its three strips, overlapping later groups' matmuls.
        with tc.tile_pool(name="ropool", bufs=3) as ropool, \
             tc.tile_pool(name="qkpool", bufs=2) as qkpool, \
             tc.tile_pool(name="apool", bufs=2) as apool, \
             tc.tile_pool(name="qkv_psum", bufs=3, space="PSUM") as psum_mm, \
             tc.tile_pool(name="at_psum", bufs=3, space="PSUM", side="right") as psum_at, \
             tc.tile_pool(name="ctx_psum", bufs=2, space="PSUM", side="right") as psum_cx:
            for g in range(4):
                qTg = qkpool.tile([128, NT, 4, 128], BF16, tag="qTg")
                kTg = qkpool.tile([128, NT, 4, 128], BF16, tag="kTg")
                v_g = qkpool.tile([128, NT, 4, 132], BF16, tag="vg")
                nc.vector.memset(v_g[:, :, :, 128:129], 1.0)
                for kind in range(3):
                    src = g + 4 * kind
                    if g == 0 and kind < 2:
                        wst = wst_pre[kind]
                    else:
                        wst = qkwpool.tile([128, NK, 512], BF16, tag="w_qkv")
                        nc.gpsimd.dma_start(wst[:], wqkv_in[src, :, :, :])
                    for t in range(NT):
                        ps = psum_mm.tile([128, 512], F32, tag="ps_qkv")
                        for kk in range(NK):
                            nc.tensor.matmul(ps[:], xqT[:, t, kk, :], wst[:, kk, :],
                                             start=(kk == 0), stop=(kk == NK - 1))
                        if kind == 2:
                            nc.scalar.copy(v_g[:, t, :, 0:128],
                                           ps[:].rearrange("p (c f) -> p c f", c=4))
                        else:
                            qsc = ropool.tile([128, 4, 128], BF16, tag="qsc")
                            nc.scalar.copy(qsc[:], ps[:].rearrange("p (c f) -> p c f", c=4))
                            p1, p2 = qsc[:, :, 0:64], qsc[:, :, 64:128]
                            cosd = cs_b[:, 0, t, :].rearrange("p (c f) -> p c f", c=4)
                            sind = cs_b[:, 1, t, :].rearrange("p (c f) -> p c f", c=4)
                            t1 = ropool.tile([128, 4, 64], F32, tag="rt1")
                            t2 = ropool.tile([128, 4, 64], F32, tag="rt2")
                            rot = ropool.tile([128, 4, 128], BF16, tag="rot")
                            nc.vector.tensor_tensor(t1[:], p1, cosd, OP.mult)
                            nc.gpsimd.tensor_tensor(t2[:], p2, sind, OP.mult)
                            nc.vector.tensor_tensor(rot[:, :, 0:64], t1[:], t2[:], OP.subtract)
                            nc.vector.tensor_tensor(t1[:], p2, cosd, OP.mult)
                            nc.gpsimd.tensor_tensor(t2[:], p1, sind, OP.mult)
                            nc.vector.tensor_tensor(rot[:, :, 64:128], t1[:], t2[:], OP.add)
                            dst = qTg if kind == 0 else kTg
                            nc.sync.dma_start_transpose(
                                dst[:, t, :, :], rot[:].rearrange("p c f -> p (c f)"))
                for blk in range(2):
                    for hl in range(4):
                        expT = [None] * 4
                        for kt in range(4):
                            qn = 512 - kt * 128
                            pss = psum_at.tile([128, 512], F32, tag="ps_sc")
                            nc.tensor.matmul(
                                pss[:, 0:qn],
                                kTg[:, blk * 4 + kt, hl, :],
                                qTg[:, blk * 4 + kt:(blk + 1) * 4, hl, :],
                                start=True, stop=True)
                            ex = apool.tile([128, 512], BF16, tag=f"expT{kt}")
                            nc.scalar.activation(ex[:, 0:qn], pss[:, 0:qn], ACTF.Exp,
                                                 scale=c_att)
                            nc.gpsimd.affine_select(
                                out=ex[:, 0:128], in_=ex[:, 0:128],
                                compare_op=OP.is_ge, fill=0.0,
                                base=0, pattern=[[1, 128]], channel_multiplier=-1)
                            expT[kt] = ex
                        for qt in range(4):
                            psc = psum_cx.tile([128, 132], F32, tag="ps_ctx")
                            for kt in range(qt + 1):
                                nc.tensor.matmul(psc[:, 0:129],
                                                 expT[kt][:, (qt - kt) * 128:(qt - kt) * 128 + 128],
                                                 v_g[:, blk * 4 + kt, hl, 0:129],
                                                 start=(kt == 0), stop=(kt == qt))
                            rl = apool.tile([128, 1], F32, tag="rl")
                            nc.vector.reciprocal(rl[:], psc[:, 128:129])
                            nc.vector.tensor_scalar_mul(
                                ctx_sb[:, blk * 4 + qt, 4 * g + hl, :],
                                psc[:, 0:128], rl[:])
        qkwpool.release()
        slot1.release()

        # ---------- P4: ctx act-qdq + transpose ----------
        ctxq_pool = tc.alloc_tile_pool(name="ctxq_pool", bufs=1, side="right")
        ctxqT = ctxq_pool.tile([128, NT, NK, 128], BF16)
        with tc.tile_pool(name="cqpool", bufs=2) as cqpool:
            for t in range(NT):
                src = ctx_sb[:, t, :, :].rearrange("p c f -> p (c f)")
                amax = cqpool.tile([128, 1], F32, tag="amax_cq")
                nc.vector.tensor_reduce(amax[:], src, AX.X, OP.max,
                                        apply_absolute_value=True)
                amc = cqpool.tile([128, 1], F32, tag="amc_cq")
                nc.vector.tensor_scalar_max(amc[:], amax[:], 1e-5)
                rec = cqpool.tile([128, 1], F32, tag="rec_cq")
                nc.vector.reciprocal(rec[:], amc[:])
                nc.vector.tensor_scalar_mul(sc[:, t:t + 1], rec[:], 127.0)
                nc.vector.tensor_scalar_mul(dqc[:, t:t + 1], amc[:], 1.0 / 127.0)
                mg = cqpool.tile([128, H], F32, tag="mg_cq")
                if t % 2 == 0:
                    nc.scalar.activation(mg[:], src, ACTF.Identity, bias=magic_ap,
                                         scale=sc[:, t:t + 1])
                else:
                    nc.vector.tensor_scalar(mg[:], src, sc[:, t:t + 1], float(MAGIC),
                                            OP.mult, OP.add)
                cq = cqpool.tile([128, H], BF16, tag="cq")
                nc.vector.tensor_scalar(cq[:], mg[:], float(MAGIC), dqc[:, t:t + 1],
                                        OP.subtract, OP.mult)
                nc.sync.dma_start_transpose(ctxqT[:, t, :, :], cq[:])
        ctx_pool.release()

        # ---------- P5: o_proj + residual -> x1_d (+ ffn-norm ssq fused) ----------
        # ---------- P6: ffn rmsnorm (slim) + transpose ----------
        hnT_pool = tc.alloc_tile_pool(name="hnT_pool", bufs=1)
        hnT = hnT_pool.tile([128, NT, NK, 128], BF16)
        upool = tc.alloc_tile_pool(name="upool", bufs=2)
        with tc.tile_pool(name="opool", bufs=3) as opool, \
             tc.tile_pool(name="owpool", bufs=2) as owpool, \
             tc.tile_pool(name="n2pool", bufs=2) as n2pool, \
             tc.tile_pool(name="n2wpool", bufs=1) as n2wpool, \
             tc.tile_pool(name="o_psum", bufs=3, space="PSUM") as psum_o:
            fnw_b = n2wpool.tile([128, H], F32, tag="normw2")
            ap2 = fnw_in[:]
            nc.gpsimd.dma_start(out=fnw_b[:], in_=bass.AP(
                tensor=ap2.tensor, offset=ap2.offset, ap=[[0, 128]] + list(ap2.ap)))
            for nn in range(4):
                wst = owpool.tile([128, NK, 512], BF16, tag="wo_st")
                nc.gpsimd.dma_start(wst[:], wo_in[nn, :, :, :])
                for t in range(NT):
                    ps = psum_o.tile([128, 512], F32, tag="ps_o")
                    for kk in range(NK):
                        nc.tensor.matmul(ps[:], ctxqT[:, t, kk, :], wst[:, kk, :],
                                         start=(kk == 0), stop=(kk == NK - 1))
                    xs = opool.tile([128, 512], F32, tag="xs")
                    nc.scalar.dma_start(xs[:], x_in[t * 128:(t + 1) * 128,
                                                    nn * 512:(nn + 1) * 512])
                    tmp = opool.tile([128, 512], F32, tag="o_tmp")
                    nc.scalar.activation(tmp[:], ps[:], ACTF.Identity, scale=c_o)
                    x1s = opool.tile([128, 512], F32, tag="x1s")
                    nc.vector.tensor_tensor(x1s[:], tmp[:], xs[:], OP.add)
                    sqt = opool.tile([128, 512], F32, tag="o_sq")
                    nc.vector.tensor_tensor(sqt[:], x1s[:], x1s[:], OP.mult)
                    nc.vector.tensor_reduce(ssq_parts[:, t, nn:nn + 1], sqt[:],
                                            AX.X, OP.add)
                    nc.scalar.dma_start(x1_d[t, :, nn * 512:(nn + 1) * 512], x1s[:])
            up_pre = []
            for j in range(2):
                w = upool.tile([128, NK, 512], BF16, tag="w_up")
                nc.gpsimd.dma_start(w[:], wup_in[8 * j, :, :, :])
                up_pre.append(w)
            for t in range(NT):
                x1_t = n2pool.tile([128, H], F32, tag="x1n")
                nc.sync.dma_start(x1_t[:], x1_d[t, :, :])
                ssq = n2pool.tile([128, 1], F32, tag="ssq2")
                nc.vector.tensor_reduce(ssq[:], ssq_parts[:, t, :], AX.X, OP.add)
                msq = n2pool.tile([128, 1], F32, tag="msq2")
                nc.vector.tensor_scalar(msq[:], ssq[:], 1.0 / H, EPS, OP.mult, OP.add)
                sd = n2pool.tile([128, 1], F32, tag="sd2")
                nc.scalar.activation(sd[:], msq[:], ACTF.Sqrt)
                rstd = n2pool.tile([128, 1], F32, tag="rstd2")
                nc.vector.reciprocal(rstd[:], sd[:])
                hn_t = n2pool.tile([128, H], BF16, tag="hn_t")
                nc.vector.tensor_scalar_mul(hn_t[:], x1_t[:], rstd[:])
                nc.gpsimd.tensor_tensor(hn_t[:], hn_t[:], fnw_b[:], OP.mult)
                nc.sync.dma_start_transpose(hnT[:, t, :, :], hn_t[:])
        ctxq_pool.release()

        # ---------- P7: ffn up + silu*val -> act_d ----------
        with tc.tile_pool(name="fpool", bufs=3) as fpool, \
             tc.tile_pool(name="up_psum", bufs=3, space="PSUM", side="right") as psum_up, \
             tc.tile_pool(name="upv_psum", bufs=3, space="PSUM", side="right") as psum_upv:
            for i in range(8):   # paired gate/val strips of 512
                if i == 0:
                    wgr, wvr = up_pre
                else:
                    wgr = upool.tile([128, NK, 512], BF16, tag="w_up")
                    nc.gpsimd.dma_start(wgr[:], wup_in[i, :, :, :])
                    wvr = upool.tile([128, NK, 512], BF16, tag="w_up")
                    nc.gpsimd.dma_start(wvr[:], wup_in[8 + i, :, :, :])
                for t in range(NT):
                    psg = psum_up.tile([128, 512], F32, tag="ps_g")
                    for kk in range(NK):
                        nc.tensor.matmul(psg[:], hnT[:, t, kk, :], wgr[:, kk, :],
                                         start=(kk == 0), stop=(kk == NK - 1))
                    psv = psum_upv.tile([128, 512], F32, tag="ps_v")
                    for kk in range(NK):
                        nc.tensor.matmul(psv[:], hnT[:, t, kk, :], wvr[:, kk, :],
                                         start=(kk == 0), stop=(kk == NK - 1))
                    sgm = fpool.tile([128, 512], F32, tag="sgm")
                    nc.scalar.activation(sgm[:], psg[:], ACTF.Sigmoid)
                    sg = fpool.tile([128, 512], F32, tag="sg")
                    nc.vector.tensor_tensor(sg[:], sgm[:], psg[:], OP.mult)
                    av = fpool.tile([128, 512], BF16, tag="av")
                    nc.vector.tensor_tensor(av[:], sg[:], psv[:], OP.mult)
                    nc.sync.dma_start(act_d[t, :, i * 512:(i + 1) * 512], av[:])
                    rmax = fpool.tile([128, 1], F32, tag="rmax")
                    nc.vector.tensor_reduce(rmax[:], av[:], AX.X, OP.max,
                                            apply_absolute_value=True)
                    nc.vector.tensor_tensor(amax_av[:, t:t + 1], amax_av[:, t:t + 1],
                                            rmax[:], OP.max)

        upool.release()
        hnT_pool.release()

        # ---------- P8: ffn act-qdq + transpose -> actqT (SBUF) ----------
        # ---------- P9: ffn down + residual -> out ----------
        actq_pool = tc.alloc_tile_pool(name="actq_pool", bufs=1)
        actqT = actq_pool.tile([128, NT, NKI, 128], BF16)
        with tc.tile_pool(name="aqpool", bufs=2) as aqpool, \
             tc.tile_pool(name="dpool", bufs=2) as dpool, \
             tc.tile_pool(name="dopool", bufs=3) as dopool, \
             tc.tile_pool(name="dn_psum", bufs=4, space="PSUM") as psum_dn:
            # prefetch first down-weight strip ahead of the quant pass
            wdn_tiles = {}
            for half in range(2):
                w = dpool.tile([128, 16, 512], BF16, tag=f"w_dn_{half}")
                nc.gpsimd.dma_start(w[:], wdn_in[0, half, :, :, :])
                wdn_tiles[(0, half)] = w
            for t in range(NT):
                amc = aqpool.tile([128, 1], F32, tag="amc_a")
                nc.vector.tensor_scalar_max(amc[:], amax_av[:, t:t + 1], 1e-5)
                rec = aqpool.tile([128, 1], F32, tag="rec_a")
                nc.vector.reciprocal(rec[:], amc[:])
                nc.vector.tensor_scalar_mul(sa[:, t:t + 1], rec[:], 127.0)
                nc.vector.tensor_scalar_mul(dqa[:, t:t + 1], amc[:], 1.0 / 127.0)
                for hf in range(4):
                    at = aqpool.tile([128, 1024], BF16, tag="at")
                    nc.scalar.dma_start(at[:], act_d[t, :, hf * 1024:(hf + 1) * 1024])
                    aq = aqpool.tile([128, 1024], BF16, tag="aq")
                    if hf % 2 == 0:
                        mg = aqpool.tile([128, 1024], F32, tag="mg_a")
                        nc.scalar.activation(mg[:], at[:], ACTF.Identity,
                                             bias=magic_ap, scale=sa[:, t:t + 1])
                        nc.vector.tensor_scalar(aq[:], mg[:], float(MAGIC),
                                                dqa[:, t:t + 1], OP.subtract, OP.mult)
                    else:
                        mg = aqpool.tile([128, 1024], F32, tag="mg_b")
                        nc.vector.tensor_scalar(mg[:], at[:], sa[:, t:t + 1],
                                                float(MAGIC), OP.mult, OP.add)
                        nc.vector.tensor_scalar(aq[:], mg[:], float(MAGIC),
                                                dqa[:, t:t + 1], OP.subtract, OP.mult)
                    nc.sync.dma_start_transpose(actqT[:, t, hf * 8:(hf + 1) * 8, :],
                                                aq[:])
            for nn in range(4):
                if nn > 0:
                    for half in range(2):
                        w = dpool.tile([128, 16, 512], BF16, tag=f"w_dn_{half}")
                        nc.gpsimd.dma_start(w[:], wdn_in[nn, half, :, :, :])
                        wdn_tiles[(nn, half)] = w
                wsa = wdn_tiles[(nn, 0)]
                wsb = wdn_tiles[(nn, 1)]
                for t in range(NT):
                    ps = psum_dn.tile([128, 512], F32, tag="ps_dn")
                    for kk in range(NKI):
                        w = wsa[:, kk, :] if kk < 16 else wsb[:, kk - 16, :]
                        nc.tensor.matmul(ps[:], actqT[:, t, kk, :], w,
                                         start=(kk == 0), stop=(kk == NKI - 1))
                    x1_t = dopool.tile([128, 512], F32, tag="x1_re")
                    nc.gpsimd.dma_start(x1_t[:], x1_d[t, :, nn * 512:(nn + 1) * 512])
                    tmp = dopool.tile([128, 512], F32, tag="d_tmp")
                    nc.vector.tensor_scalar_mul(tmp[:], ps[:], c_dn)
                    ot = dopool.tile([128, 512], F32, tag="ot")
                    nc.vector.tensor_tensor(ot[:], tmp[:], x1_t[:], OP.add)
                    nc.sync.dma_start(out_d[t * 128:(t + 1) * 128,
                                            nn * 512:(nn + 1) * 512], ot[:])
        actq_pool.release()
        cs_pool.release()
        perm.release()

    nc.compile()
    return nc


_NC_CACHE = None


def _get_nc():
    global _NC_CACHE
    if _NC_CACHE is None:
        _NC_CACHE = build_program()
    return _NC_CACHE


def _ternarize(w):
    """Reference _weight_quant: returns (ternary float {-1,0,1}, dqw scale)."""
    w = np.asarray(w, np.float32)
    m = np.maximum(np.mean(np.abs(w), dtype=np.float32), np.float32(1e-5))
    scale = np.float32(1.0) / m
    tern = np.clip(np.round(w * scale), -1.0, 1.0).astype(np.float32)
    return tern, float(m)


def _strip_layout(w_t, n_strips, nk):
    """[in_feats, out_feats] -> [n_strips, 128, nk, 512] (strip s covers out
    cols s*512..). w_t is the transposed weight [in, out]."""
    infeat, outfeat = w_t.shape
    assert infeat == nk * 128 and outfeat == n_strips * 512
    # [nk, 128, n_strips, 512] -> [n_strips, 128, nk, 512]
    v = w_t.reshape(nk, 128, n_strips, 512)
    return np.ascontiguousarray(v.transpose(2, 1, 0, 3))


def _host_inputs(x, attn_norm_w, ffn_norm_w, qkv_w, o_w, ffn_up_w, ffn_down_w):
    x = np.ascontiguousarray(np.asarray(x, np.float32))
    anw = np.ascontiguousarray(np.asarray(attn_norm_w, np.float32))
    fnw = np.ascontiguousarray(np.asarray(ffn_norm_w, np.float32))

    tern_qkv, dqw_qkv = _ternarize(qkv_w)
    tern_o, dqw_o = _ternarize(o_w)
    tern_dn, dqw_dn = _ternarize(ffn_down_w)

    wqkv_sh = _strip_layout(tern_qkv.T, 12, NK).astype(ml_dtypes.bfloat16)
    wo_sh = _strip_layout(tern_o.T, 4, NK).astype(ml_dtypes.bfloat16)
    wup_sh = _strip_layout(np.asarray(ffn_up_w, np.float32).T, 16, NK) \
        .astype(ml_dtypes.bfloat16)
    wdn_sh = _strip_layout(tern_dn.T, 4, NKI).astype(ml_dtypes.bfloat16) \
        .reshape(4, 128, 2, 16, 512).transpose(0, 2, 1, 3, 4)
    wdn_sh = np.ascontiguousarray(wdn_sh)

    consts = np.zeros(8, np.float32)
    consts[0] = dqw_qkv * dqw_qkv * (HD ** -0.5)
    consts[1] = dqw_qkv * dqw_o
    consts[2] = dqw_dn

    # rope tables: cs[p, 0/1, t, 4*64] bf16, replicated x4 for the 4-head strips
    inv = 1.0 / (THETA ** (np.arange(0, HD, 2, dtype=np.float32) / HD))
    tpos = np.arange(S, dtype=np.float32)
    fr = np.outer(tpos, inv)                     # [S, 64]
    cos = np.tile(np.cos(fr), (1, 4))            # [S, 256]
    sin = np.tile(np.sin(fr), (1, 4))

    in_maps = []
    for c in range(NCORES):
        b = c // 4
        t0 = (c % 4) * R
        # [R, 256] -> [NT, 128, 256] -> [128, NT, 256]
        cs = np.stack([cos[t0:t0 + R].reshape(NT, 128, 256).transpose(1, 0, 2),
                       sin[t0:t0 + R].reshape(NT, 128, 256).transpose(1, 0, 2)],
                      axis=1)                    # [128, 2, NT, 256]
        in_maps.append({
            "x_sh": np.ascontiguousarray(x[b, t0:t0 + R, :]),
            "cs_sh": np.ascontiguousarray(cs).astype(ml_dtypes.bfloat16),
            "attn_norm_w": anw, "ffn_norm_w": fnw, "consts": consts,
            "wqkv_sh": wqkv_sh, "wo_sh": wo_sh, "wup_sh": wup_sh, "wdn_sh": wdn_sh,
        })
    return in_maps


def run(trace=False, **inputs):
    nc = _get_nc()
    in_maps = _host_inputs(**inputs)
    res = run_bass_kernel_spmd(nc, in_maps, list(range(NCORES)), trace=trace)
    out = np.empty((B, S, H), np.float32)
    for c in range(NCORES):
        b = c // 4
        t0 = (c % 4) * R
        out[b, t0:t0 + R, :] = res.results[c]["out_sh"]
    return out, res


def kernel(**inputs):
    out, _ = run(trace=False, **inputs)
    return out
